# revision 41
# baseline (speedup 1.0000x reference)
"""Trainium2 Bass kernel for CausalCrossAttention (B=8, T=769, C=1024, H=16).

Sharding: data-parallel over batch B=8 across the 8 NeuronCores (one batch
element per core, SPMD).

v2 (vs the fp32r baseline): all matmul operands in bf16 (fp32 PSUM accum),
DMA ring hygiene (bulk loads on sync+gpsimd rings, dependent SBUF swaps on
the scalar ring), per-projection weight tags so the next matrix prefetches
during the current projection, PE warmup matmuls under the initial DMAs,
V-projection overlapped with the first two head-pairs' S^T+exp (ACT head
start), per-nk interleaved attention (S^T a/b, exp, PV a/b) with an exact
8-bank PSUM budget, and an on-chip softmax denominator path
(DVE reciprocal + GpSimd partition_broadcast — no DRAM bounce).
"""

import os

import numpy as np

B, T, C = 8, 769, 1024
H, HD, L = 16, 64, 32
COND = 256
NCI = 8  # 1024 / 128 contraction tiles
NCO = 8
NTT = 7  # t tiles: 6 full + 1 single row
TP = 770  # streamed T padded to even
R0 = (0, 512)
R1 = (512, 770)
VW = H * (HD + 1) + 63  # V_aug free width

_CACHE = {}


def _build_program():
    import concourse.mybir as mybir
    import concourse.tile as tile
    from concourse import bacc

    f32 = mybir.dt.float32
    f32r = mybir.dt.float32r
    bf = mybir.dt.bfloat16
    Exp = mybir.ActivationFunctionType.Exp
    Ident = mybir.ActivationFunctionType.Identity

    nc = bacc.Bacc("TRN2", target_bir_lowering=False)

    xqT_d = nc.dram_tensor("xqT", [C, TP], bf, kind="ExternalInput")
    xkvT_d = nc.dram_tensor("xkvT", [C, TP], bf, kind="ExternalInput")
    wq_d = nc.dram_tensor("wqT", [C, C], bf, kind="ExternalInput")
    wk_d = nc.dram_tensor("wkT", [C, C], bf, kind="ExternalInput")
    wv_d = nc.dram_tensor("wvT", [C, C], bf, kind="ExternalInput")
    wp_d = nc.dram_tensor("wpT", [C, C], bf, kind="ExternalInput")
    bq_d = nc.dram_tensor("bq2", [128, NCO], f32, kind="ExternalInput")
    bk_d = nc.dram_tensor("bk2", [128, NCO], f32, kind="ExternalInput")
    bv_d = nc.dram_tensor("bv1", [1, C], f32, kind="ExternalInput")
    bp_d = nc.dram_tensor("bp1", [1, C], f32, kind="ExternalInput")
    cos_d = nc.dram_tensor("cosP", [128, TP], bf, kind="ExternalInput")
    sin_d = nc.dram_tensor("sinP", [128, TP], bf, kind="ExternalInput")
    m0_d = nc.dram_tensor("m0", [128, 128], bf, kind="ExternalInput")
    out_d = nc.dram_tensor("out", [T, C], f32, kind="ExternalOutput")

    # Per-(kv-tile) q ranges in the 0:512 block + mask offset.
    R0SUB = {0: (0, 512, None), 1: (0, 512, None), 2: (0, 512, 0),
             3: (128, 512, 128), 4: (256, 512, 256), 5: (384, 512, 384)}

    with tile.TileContext(nc) as tc:
        with (
            tc.tile_pool(name="consts", bufs=1) as consts,
            tc.tile_pool(name="wpool", bufs=1) as wpool,
            tc.tile_pool(name="qkpool", bufs=1) as qkpool,
            tc.tile_pool(name="vpool", bufs=1) as vpool,
        ):
            cos_sb = consts.tile([128, TP], bf, tag="cos")
            sin_sb = consts.tile([128, TP], bf, tag="sin")
            m0_sb = consts.tile([128, 128], bf, tag="m0")
            bq_sb = consts.tile([128, NCO], f32, tag="bq")
            bk_sb = consts.tile([128, NCO], f32, tag="bk")
            ones16 = consts.tile([128, 16], f32, tag="ones16")
            nc.vector.memset(ones16, 1.0)
            zbf = consts.tile([128, TP], bf, tag="zbf")
            nc.vector.memset(zbf, 0.0)
            wz = consts.tile([128, 128], bf, tag="wz")
            nc.vector.memset(wz, 0.0)
            ones128 = consts.tile([128, 128], bf, tag="ones128")
            nc.vector.memset(ones128, 1.0)
            nc.scalar.dma_start(out=cos_sb, in_=cos_d[:, :])
            nc.scalar.dma_start(out=sin_sb, in_=sin_d[:, :])
            nc.scalar.dma_start(out=m0_sb, in_=m0_d[:, :])
            nc.scalar.dma_start(out=bq_sb, in_=bq_d[:, :])
            nc.scalar.dma_start(out=bk_sb, in_=bk_d[:, :])

            qT = qkpool.tile([128, NCI, TP], bf, tag="qT")
            kT = qkpool.tile([128, NCI, TP], bf, tag="kT")
            vaug = vpool.tile([128, NTT, VW], bf, tag="vaug")

            def load_w(wdram, engine, tagpfx, name):
                # one DMA per 1 MB half — per-dma_start fixed cost would
                # otherwise dominate the load latency
                halves = []
                for hf in (0, 1):
                    wt = wpool.tile([128, NCI, 512], bf,
                                    tag=f"{tagpfx}h{hf}",
                                    name=f"{name}h{hf}")
                    engine.dma_start(
                        out=wt,
                        in_=wdram[:, hf * 512:(hf + 1) * 512].rearrange(
                            "(ci p) f -> p ci f", p=128))
                    halves.append(wt)
                return [[halves[0][:, ci, :], halves[1][:, ci, :]]
                        for ci in range(NCI)]

            def proj_qk(w, x, b_sb, outT, psA, shpool, filler=None):
                """outT[:, co, :] = W @ x^T + b, then partial rotary.

                Groups of 2 co so accumulating + draining PSUM tiles
                double-buffer; rotary swap DMAs batched per group on the
                sync ring (the ACT queue stays pure compute).
                """
                for g in range(4):
                    c0 = 2 * g
                    pss = [psA.tile([128, 1024], f32, tag="ps",
                                    name=f"ps{c0 + k}") for k in (0, 1)]
                    for ci in range(NCI):
                        for k in (0, 1):
                            co = c0 + k
                            lhs = w[ci][co // 4][
                                :, (co % 4) * 128:(co % 4 + 1) * 128]
                            for (lo, hi) in (R0, R1):
                                nc.tensor.matmul(
                                    pss[k][:, lo:hi], lhs, x[:, ci, lo:hi],
                                    start=(ci == 0), stop=(ci == NCI - 1))
                        if filler is not None:
                            filler(g, ci)
                    for k in (0, 1):
                        nc.scalar.activation(
                            out=outT[:, c0 + k, :], in_=pss[k][:, 0:TP],
                            func=Ident, bias=b_sb[:, c0 + k:c0 + k + 1],
                            scale=1.0)
                    sh = shpool.tile([128, 2, TP], bf, tag="sh",
                                     name=f"sh{c0}")
                    nc.sync.dma_start(
                        out=sh[32:64, :, :], in_=outT[32:64, c0:c0 + 2, :])
                    for s in (0, 64):
                        nc.sync.dma_start(
                            out=sh[s:s + 16, :, :],
                            in_=outT[s + 16:s + 32, c0:c0 + 2, :])
                        nc.sync.dma_start(
                            out=sh[s + 16:s + 32, :, :],
                            in_=outT[s:s + 16, c0:c0 + 2, :])
                    for k in (0, 1):
                        co = c0 + k
                        nc.vector.tensor_mul(
                            sh[0:96, k, :], sh[0:96, k, :], sin_sb[0:96, :])
                        nc.vector.tensor_mul(
                            outT[:, co, :], outT[:, co, :], cos_sb)
                        nc.vector.tensor_add(
                            outT[0:96, co, :], outT[0:96, co, :],
                            sh[0:96, k, :])

            # ---- phase 1: Q/K projections ----
            with (
                tc.tile_pool(name="xkv", bufs=1) as xkp,
                tc.tile_pool(name="shpool", bufs=2) as shpool,
            ):
                xkv = xkp.tile([128, NCI, TP], bf, tag="xkv")
                bv_sb = xkp.tile([128, C], f32, tag="bv")
                with (
                    tc.tile_pool(name="psA", bufs=3, space="PSUM") as psA,
                    tc.tile_pool(name="psW", bufs=1, space="PSUM") as psW,
                    tc.tile_pool(name="xq", bufs=1) as xqp,
                ):
                    # PE warmup: keep the activity monitor busy while the
                    # first weight/x DMAs land, so matmuls run at 2.4 GHz.
                    pw = psW.tile([128, 512], f32, tag="pw")
                    for _ in range(44):
                        nc.tensor.matmul(pw[:, 0:128], wz, wz,
                                         start=True, stop=True)

                    def warm_filler(g, ci):
                        # filler matmuls woven into early Q-proj emission:
                        # they run whenever the PE would otherwise idle on
                        # a DMA wait, keeping the HAM clock at 8/8.
                        n = 10 if g == 0 else (4 if g == 1 else 0)
                        for _ in range(n):
                            nc.tensor.matmul(pw[:, 0:128], wz, wz,
                                             start=True, stop=True)

                    xq = xqp.tile([128, NCI, TP], bf, tag="xq")
                    # sync ring: wq h0, xq, wq h1 (each one big DMA);
                    # scalar ring: wk h0 (behind the small consts);
                    # gpsimd ring: xkv, wk h1, bv
                    wqh0 = wpool.tile([128, NCI, 512], bf, tag="wAh0",
                                      name="wqh0")
                    nc.sync.dma_start(
                        out=wqh0,
                        in_=wq_d[:, 0:512].rearrange(
                            "(ci p) f -> p ci f", p=128))
                    nc.sync.dma_start(
                        out=xq,
                        in_=xqT_d.rearrange("(ci p) f -> p ci f", p=128))
                    wqh1 = wpool.tile([128, NCI, 512], bf, tag="wAh1",
                                      name="wqh1")
                    nc.sync.dma_start(
                        out=wqh1,
                        in_=wq_d[:, 512:1024].rearrange(
                            "(ci p) f -> p ci f", p=128))
                    wq = [[wqh0[:, ci, :], wqh1[:, ci, :]]
                          for ci in range(NCI)]
                    wkh0 = wpool.tile([128, NCI, 512], bf, tag="wBh0",
                                      name="wkh0")
                    nc.scalar.dma_start(
                        out=wkh0,
                        in_=wk_d[:, 0:512].rearrange(
                            "(ci p) f -> p ci f", p=128))
                    nc.gpsimd.dma_start(
                        out=xkv,
                        in_=xkvT_d.rearrange("(ci p) f -> p ci f", p=128))
                    wkh1 = wpool.tile([128, NCI, 512], bf, tag="wBh1",
                                      name="wkh1")
                    nc.gpsimd.dma_start(
                        out=wkh1,
                        in_=wk_d[:, 512:1024].rearrange(
                            "(ci p) f -> p ci f", p=128))
                    wk = [[wkh0[:, ci, :], wkh1[:, ci, :]]
                          for ci in range(NCI)]
                    nc.gpsimd.dma_start(
                        out=bv_sb, in_=bv_d[0:1, :].broadcast_to((128, C)))

                    proj_qk(wq, xq, bq_sb, qT, psA, shpool,
                            filler=warm_filler)
                    # wv prefetch on sync ring (lands during Q/K compute)
                    wv = load_w(wv_d, nc.sync, "wV", "wv")
                    proj_qk(wk, xkv, bk_sb, kT, psA, shpool)

                # ---- phase 2+3 ----
                with tc.tile_pool(name="ypool", bufs=1) as ypool:
                  yT = ypool.tile([128, NCI, TP], bf, tag="yT")
                  with (
                    tc.tile_pool(name="ptp", bufs=2) as ptpool,
                    tc.tile_pool(name="qzp", bufs=1) as qzpool,
                    tc.tile_pool(name="psS", bufs=2, space="PSUM") as psS,
                    tc.tile_pool(name="stgp", bufs=2) as stgpool,
                    tc.tile_pool(name="rdp", bufs=1) as rdpool,
                    tc.tile_pool(name="ycp", bufs=2) as ycpool,
                  ):
                    # wp reuses wq's tags/buffers (dead after Q-proj)
                    wp = load_w(wp_d, nc.sync, "wA", "wp")
                    bp_sb = ypool.tile([128, C], f32, tag="bp")
                    nc.gpsimd.dma_start(
                        out=bp_sb, in_=bp_d[0:1, :].broadcast_to((128, C)))


                    qza = [qzpool.tile([128, TP], bf, tag=f"qza{i}",
                                       name=f"qza{i}")
                           for i in (0, 1)]
                    qzb = [qzpool.tile([128, TP], bf, tag=f"qzb{i}",
                                       name=f"qzb{i}")
                           for i in (0, 1)]
                    for i in (0, 1):
                        nc.vector.memset(qza[i][64:128, :], 0.0)
                        nc.vector.memset(qzb[i][0:64, :], 0.0)

                    def emit_qz(j):
                        nc.vector.tensor_copy(
                            qza[j % 2][0:64, :], qT[0:64, j, :])
                        nc.vector.tensor_copy(
                            qzb[j % 2][64:128, :], qT[64:128, j, :])

                    pts = {h: {} for h in range(H)}

                    def emit_st(h, nk):
                        j, par = h // 2, h % 2
                        qzt = qza[j % 2] if par == 0 else qzb[j % 2]
                        qlo, qhi, moff = R0SUB[nk]
                        ps = psS.tile([128, 1024], f32, tag="st",
                                      name=f"st{h}_{nk}")
                        nc.tensor.matmul(
                            ps[:, qlo:qhi],
                            kT[:, j, nk * 128:(nk + 1) * 128],
                            qzt[:, qlo:qhi], start=True, stop=True)
                        nc.tensor.matmul(
                            ps[:, 512:770],
                            kT[:, j, nk * 128:(nk + 1) * 128],
                            qzt[:, 512:770], start=True, stop=True)
                        pt = ptpool.tile([128, 772], bf, tag=f"pt{par}{nk}",
                                         name=f"pt{h}_{nk}")
                        nc.scalar.activation(
                            out=pt[:, qlo:770], in_=ps[:, qlo:770],
                            func=Exp, scale=0.125)
                        if moff is not None:
                            nc.gpsimd.tensor_mul(
                                pt[:, moff:moff + 128],
                                pt[:, moff:moff + 128], m0_sb)
                        pts[h][nk] = pt

                    def emit_st_tail(h):
                        j, par = h // 2, h % 2
                        qzt = qza[j % 2] if par == 0 else qzb[j % 2]
                        ps = psS.tile([128, 1024], f32, tag="st",
                                      name=f"st{h}_6")
                        nc.tensor.matmul(
                            ps[0:1, 0:258], kT[:, j, 768:769],
                            qzt[:, 512:770], start=True, stop=True)
                        pt6 = ptpool.tile([1, 772], bf, tag=f"pt{par}6",
                                          name=f"pt6_{h}")
                        nc.scalar.activation(
                            out=pt6[0:1, 513:770], in_=ps[0:1, 1:258],
                            func=Exp, scale=0.125)
                        nc.vector.tensor_copy(
                            pt6[0:1, 512:513], zbf[0:1, 0:1])
                        pts[h][6] = pt6

                    def emit_pv(h, nk, o):
                        vs = slice(h * (HD + 1), h * (HD + 1) + 128)
                        qlo, qhi, _ = R0SUB[nk]
                        nc.tensor.matmul(
                            o[:, qlo:qhi], vaug[:, nk, vs],
                            pts[h][nk][:, qlo:qhi],
                            start=(nk == 0), stop=False)
                        nc.tensor.matmul(
                            o[:, 512:770], vaug[:, nk, vs],
                            pts[h][nk][:, 512:770],
                            start=(nk == 0), stop=False)

                    def emit_pv_tail(h, o):
                        vs = slice(h * (HD + 1), h * (HD + 1) + 128)
                        nc.tensor.matmul(
                            o[:, 512:770], vaug[0:1, 6, vs],
                            pts[h][6][0:1, 512:770],
                            start=False, stop=True)

                    def emit_div_a(j, oa, ob):
                        # copy numerators+denominator rows out of PSUM in
                        # one wide DVE op per head — this alone releases
                        # psO for the next pair; everything downstream is
                        # off the PE's critical path.
                        yca = ycpool.tile([65, TP], bf, tag="yca",
                                          name=f"yca{j}")
                        ycb = ycpool.tile([65, TP], bf, tag="ycb",
                                          name=f"ycb{j}")
                        nc.vector.tensor_copy(yca[0:65, :], oa[0:65, 0:770])
                        nc.vector.tensor_copy(ycb[0:65, :], ob[0:65, 0:770])
                        return yca, ycb

                    def emit_div_b(j, yca, ycb):
                        # ones-row K=1 matmul broadcasts the raw denominator
                        # across 64 partitions; reciprocal on 64 DVE lanes
                        # straight out of PSUM; then the two division muls.
                        pba = psS.tile([128, 1024], f32, tag="st",
                                       name=f"pba{j}")
                        pbb = psS.tile([128, 1024], f32, tag="st",
                                       name=f"pbb{j}")
                        for (lo, hi) in (R0, R1):
                            nc.tensor.matmul(pba[0:64, lo:hi],
                                             ones128[64:65, 0:64],
                                             yca[64:65, lo:hi],
                                             start=True, stop=True)
                            nc.tensor.matmul(pbb[0:64, lo:hi],
                                             ones128[64:65, 0:64],
                                             ycb[64:65, lo:hi],
                                             start=True, stop=True)
                        ra = rdpool.tile([64, TP], f32, tag="ra",
                                         name=f"ra{j}")
                        rb = rdpool.tile([64, TP], f32, tag="rb",
                                         name=f"rb{j}")
                        nc.vector.reciprocal_approx_fast(
                            out=ra, in_=pba[0:64, 0:770])
                        nc.vector.reciprocal_approx_fast(
                            out=rb, in_=pbb[0:64, 0:770])
                        nc.vector.tensor_mul(
                            yT[0:64, j, :], yca[0:64, :], ra)
                        nc.vector.tensor_mul(
                            yT[64:128, j, :], ycb[0:64, :], rb)

                    emit_qz(0)
                    emit_qz(1)

                    # ---- V projection, interleaved with pair 0/1 S^T+exp
                    # (ACT gets a head start while the PE does V) ----
                    pre_units = []
                    for h in (0, 1, 2, 3):
                        for nk in range(6):
                            pre_units.append((h, nk))
                        pre_units.append((h, 6))

                    def emit_vgroup(g, wv):
                        pss = {}
                        for tt in g:
                            pss[tt] = psV.tile([128, 1024], f32, tag="psv",
                                               name=f"psv{tt}")
                        for ci in range(NCI):
                            for tt in g:
                                tsz = 128 if tt < 6 else 1
                                lhs = xkv[:, ci, tt * 128:tt * 128 + tsz]
                                for hf in (0, 1):
                                    nc.tensor.matmul(
                                        pss[tt][:tsz,
                                                hf * 512:hf * 512 + 512],
                                        lhs, wv[ci][hf],
                                        start=(ci == 0),
                                        stop=(ci == NCI - 1))
                        for tt in g:
                            tsz = 128 if tt < 6 else 1
                            va = vaug[:tsz, tt, 0:H * (HD + 1)].rearrange(
                                "p (h e) -> p h e", e=HD + 1)
                            nc.vector.tensor_add(
                                va[:, :, 0:HD],
                                pss[tt][:tsz, :].rearrange(
                                    "p (h d) -> p h d", h=H),
                                bv_sb[:tsz, :].rearrange(
                                    "p (h d) -> p h d", h=H))
                            nc.vector.tensor_copy(
                                va[:, :, HD:HD + 1],
                                ones16[:tsz, :].unsqueeze(2))
                            nc.vector.tensor_copy(
                                vaug[:tsz, tt, H * (HD + 1):VW],
                                zbf[:tsz, 0:VW - H * (HD + 1)])

                    with tc.tile_pool(name="psV", bufs=2,
                                      space="PSUM") as psV:
                        vgroups = [(0, 1), (2, 3), (4, 5), (6,)]
                        ui = 0
                        for gi, g in enumerate(vgroups):
                            emit_vgroup(g, wv)
                            n_units = 7 * (gi + 1)
                            while ui < min(n_units, len(pre_units)):
                                h, nk = pre_units[ui]
                                if nk == 6:
                                    emit_st_tail(h)
                                else:
                                    emit_st(h, nk)
                                ui += 1
                        while ui < len(pre_units):
                            h, nk = pre_units[ui]
                            if nk == 6:
                                emit_st_tail(h)
                            else:
                                emit_st(h, nk)
                            ui += 1

                    # ---- attention pairs ----
                    with tc.tile_pool(name="psO", bufs=1,
                                      space="PSUM") as psO:
                        pending = None  # previous pair's deferred division
                        for j in range(NCI):
                            a, bh = 2 * j, 2 * j + 1
                            oa = psO.tile([128, 1024], f32, tag="ova",
                                          name=f"ov{a}")
                            ob = psO.tile([128, 1024], f32, tag="ovb",
                                          name=f"ov{bh}")
                            for nk in range(6):
                                # S^T one nk ahead of PV so the exp+mask
                                # chain has a full nk of slack
                                if j >= 2:
                                    if nk < 5:
                                        emit_st(a, nk + 1)
                                        emit_st(bh, nk + 1)
                                    else:
                                        emit_st_tail(a)
                                        emit_st_tail(bh)
                                if nk == 2 and pending is not None:
                                    emit_div_b(*pending)
                                    pending = None
                                emit_pv(a, nk, oa)
                                emit_pv(bh, nk, ob)
                            # cross-pair lookahead: next pair's first S^T
                            # before this pair's tails, so ACT never idles
                            # across the pair boundary
                            if j + 1 >= 2 and j + 1 < NCI:
                                emit_st(a + 2, 0)
                                emit_st(bh + 2, 0)
                            emit_pv_tail(a, oa)
                            emit_pv_tail(bh, ob)
                            yca, ycb = emit_div_a(j, oa, ob)
                            pending = (j, yca, ycb)
                            # lookahead qz AFTER this pair's S^T emissions
                            # (correct WAR/RAW) and after the psO-releasing
                            # copies (DVE queue priority)
                            if j + 2 < NCI:
                                emit_qz(j + 2)
                        emit_div_b(*pending)

                  # ---- phase 3: output projection ----
                  with (
                    tc.tile_pool(name="psF", bufs=4, space="PSUM") as psF,
                    tc.tile_pool(name="opool", bufs=3) as opool,
                  ):
                    for g in (range(0, 4), range(4, 7)):
                        pss = {}
                        for tt in g:
                            pss[tt] = psF.tile([128, 1024], f32, tag="pso",
                                               name=f"pso{tt}")
                        for ci in range(NCI):
                            for tt in g:
                                tsz = 128 if tt < 6 else 1
                                lhs = yT[:, ci, tt * 128:tt * 128 + tsz]
                                for hf in (0, 1):
                                    nc.tensor.matmul(
                                        pss[tt][:tsz,
                                                hf * 512:hf * 512 + 512],
                                        lhs, wp[ci][hf],
                                        start=(ci == 0),
                                        stop=(ci == NCI - 1))
                        for tt in g:
                            tsz = 128 if tt < 6 else 1
                            ot = opool.tile([128, 1024], f32, tag="ot",
                                            name="ot")
                            nc.vector.tensor_add(
                                ot[:tsz, :], pss[tt][:tsz, :],
                                bp_sb[:tsz, :])
                            eng = nc.sync if tt % 2 == 0 else nc.scalar
                            eng.dma_start(
                                out=out_d[tt * 128:tt * 128 + tsz, :],
                                in_=ot[:tsz, :])

    nc.compile()
    return nc


def _host_prep(x_q, x_kv, rotary_pos_emb, Wq, bq, Wk, bk, Wv, bv, Wp, bp):
    import ml_dtypes
    f = np.float32
    bft = ml_dtypes.bfloat16
    x_q = np.asarray(x_q, f)
    x_kv = np.asarray(x_kv, f)
    freqs = np.asarray(rotary_pos_emb, f)

    # Even/odd pair-split permutation of the first 32 dims of each head, so
    # rotate_half becomes a 16-partition block swap on chip.
    perm = np.arange(C)
    for h in range(H):
        b0 = h * HD
        blk = np.empty(HD, np.int64)
        blk[0:16] = b0 + np.arange(0, 32, 2)
        blk[16:32] = b0 + np.arange(1, 32, 2)
        blk[32:64] = b0 + np.arange(32, 64)
        perm[b0:b0 + HD] = blk

    def wT(W, p=None):
        W = np.asarray(W, f)
        if p is not None:
            W = W[p, :]
        return np.ascontiguousarray(W.T).astype(bft)

    cosE = np.cos(freqs[:, 0::2]).T  # [16, T]
    cosO = np.cos(freqs[:, 1::2]).T
    sinE = -np.sin(freqs[:, 0::2]).T
    sinO = np.sin(freqs[:, 1::2]).T
    cosP = np.ones((128, TP), f)
    sinP = np.zeros((128, TP), f)
    for s in (0, 64):
        cosP[s:s + 16, :T] = cosE
        cosP[s + 16:s + 32, :T] = cosO
        sinP[s:s + 16, :T] = sinE
        sinP[s + 16:s + 32, :T] = sinO

    p_idx = np.arange(128)[:, None]
    f_idx = np.arange(128)[None, :]
    m0 = (p_idx < f_idx).astype(f)

    bqp = np.asarray(bq, f)[perm]
    bkp = np.asarray(bk, f)[perm]
    shared = {
        "wqT": wT(Wq, perm),
        "wkT": wT(Wk, perm),
        "wvT": wT(Wv),
        "wpT": wT(Wp),
        "bq2": np.ascontiguousarray(bqp.reshape(NCO, 128).T),
        "bk2": np.ascontiguousarray(bkp.reshape(NCO, 128).T),
        "bv1": np.asarray(bv, f).reshape(1, C).copy(),
        "bp1": np.asarray(bp, f).reshape(1, C).copy(),
        "cosP": np.ascontiguousarray(cosP).astype(bft),
        "sinP": np.ascontiguousarray(sinP).astype(bft),
        "m0": np.ascontiguousarray(m0).astype(bft),
    }

    def padT(xt):
        out = np.zeros((C, TP), f)
        out[:, :T] = xt
        return out.astype(bft)

    in_maps = []
    for b in range(B):
        m = dict(shared)
        m["xqT"] = padT(x_q[b].T)
        m["xkvT"] = padT(x_kv[b].T)
        in_maps.append(m)
    return in_maps


def kernel(x_q, x_kv, rotary_pos_emb, Wq, bq, Wk, bk, Wv, bv, Wp, bp):
    from concourse.bass_utils import run_bass_kernel_spmd

    if "nc" not in _CACHE:
        _CACHE["nc"] = _build_program()
    nc = _CACHE["nc"]

    in_maps = _host_prep(x_q, x_kv, rotary_pos_emb,
                         Wq, bq, Wk, bk, Wv, bv, Wp, bp)
    trace = os.environ.get("BTK_TRACE", "0") == "1"
    res = run_bass_kernel_spmd(
        nc, in_maps, core_ids=list(range(B)), trace=trace)
    _CACHE["last_result"] = res
    return np.stack([r["out"] for r in res.results], axis=0)


# revision 48
# speedup vs baseline: 1.0196x; 1.0196x over previous
"""Trainium2 Bass kernel for CausalCrossAttention (B=8, T=769, C=1024, H=16).

Sharding: data-parallel over batch B=8 across the 8 NeuronCores (one batch
element per core, SPMD).

v2 (vs the fp32r baseline): all matmul operands in bf16 (fp32 PSUM accum),
DMA ring hygiene (bulk loads on sync+gpsimd rings, dependent SBUF swaps on
the scalar ring), per-projection weight tags so the next matrix prefetches
during the current projection, PE warmup matmuls under the initial DMAs,
V-projection overlapped with the first two head-pairs' S^T+exp (ACT head
start), per-nk interleaved attention (S^T a/b, exp, PV a/b) with an exact
8-bank PSUM budget, and an on-chip softmax denominator path
(DVE reciprocal + GpSimd partition_broadcast — no DRAM bounce).
"""

import os

import numpy as np

B, T, C = 8, 769, 1024
H, HD, L = 16, 64, 32
COND = 256
NCI = 8  # 1024 / 128 contraction tiles
NCO = 8
NTT = 7  # t tiles: 6 full + 1 single row
TP = 770  # streamed T padded to even
R0 = (0, 512)
R1 = (512, 770)
VW = H * (HD + 1) + 63  # V_aug free width

_CACHE = {}


def _build_program():
    import concourse.mybir as mybir
    import concourse.tile as tile
    from concourse import bacc

    f32 = mybir.dt.float32
    f32r = mybir.dt.float32r
    bf = mybir.dt.bfloat16
    Exp = mybir.ActivationFunctionType.Exp
    Ident = mybir.ActivationFunctionType.Identity

    nc = bacc.Bacc("TRN2", target_bir_lowering=False)

    # all bulk inputs pre-arranged on host to [128 partitions, ...] so every
    # DMA is one fully-contiguous-per-partition transfer at line rate
    xqT_d = nc.dram_tensor("xqT", [128, NCI, TP], bf, kind="ExternalInput")
    xkvT_d = nc.dram_tensor("xkvT", [128, NCI, TP], bf,
                            kind="ExternalInput")
    w_d = {}
    for wn in ("wq", "wk", "wv", "wp"):
        for hf in (0, 1):
            w_d[wn, hf] = nc.dram_tensor(f"{wn}T{hf}", [128, NCI, 512], bf,
                                         kind="ExternalInput")
    bq_d = nc.dram_tensor("bq2", [128, NCO], f32, kind="ExternalInput")
    bk_d = nc.dram_tensor("bk2", [128, NCO], f32, kind="ExternalInput")
    bv_d = nc.dram_tensor("bv1", [1, C], f32, kind="ExternalInput")
    bp_d = nc.dram_tensor("bp1", [1, C], f32, kind="ExternalInput")
    cos_d = nc.dram_tensor("cosP", [128, TP], bf, kind="ExternalInput")
    sin_d = nc.dram_tensor("sinP", [128, TP], bf, kind="ExternalInput")
    m0_d = nc.dram_tensor("m0", [128, 128], bf, kind="ExternalInput")
    out_d = nc.dram_tensor("out", [T, C], f32, kind="ExternalOutput")

    # Per-(kv-tile) q ranges in the 0:512 block + mask offset.
    R0SUB = {0: (0, 512, None), 1: (0, 512, None), 2: (0, 512, 0),
             3: (128, 512, 128), 4: (256, 512, 256), 5: (384, 512, 384)}

    with tile.TileContext(nc) as tc:
        with (
            tc.tile_pool(name="consts", bufs=1) as consts,
            tc.tile_pool(name="wpool", bufs=1) as wpool,
            tc.tile_pool(name="qkpool", bufs=1) as qkpool,
            tc.tile_pool(name="vpool", bufs=1) as vpool,
        ):
            cos_sb = consts.tile([128, TP], bf, tag="cos")
            sin_sb = consts.tile([128, TP], bf, tag="sin")
            m0_sb = consts.tile([128, 128], bf, tag="m0")
            bq_sb = consts.tile([128, NCO], f32, tag="bq")
            bk_sb = consts.tile([128, NCO], f32, tag="bk")
            ones16 = consts.tile([128, 16], f32, tag="ones16")
            nc.vector.memset(ones16, 1.0)
            zbf = consts.tile([128, TP], bf, tag="zbf")
            nc.vector.memset(zbf, 0.0)
            wz = consts.tile([128, 128], bf, tag="wz")
            nc.vector.memset(wz, 0.0)
            ones128 = consts.tile([128, 128], bf, tag="ones128")
            nc.vector.memset(ones128, 1.0)
            nc.scalar.dma_start(out=cos_sb, in_=cos_d[:, :])
            nc.scalar.dma_start(out=sin_sb, in_=sin_d[:, :])
            nc.scalar.dma_start(out=m0_sb, in_=m0_d[:, :])
            nc.scalar.dma_start(out=bq_sb, in_=bq_d[:, :])
            nc.scalar.dma_start(out=bk_sb, in_=bk_d[:, :])

            qT = qkpool.tile([128, NCI, TP], bf, tag="qT")
            kT = qkpool.tile([128, NCI, TP], bf, tag="kT")
            vaug = vpool.tile([128, NTT, VW], bf, tag="vaug")

            def load_w(wn, engine, tagpfx, name):
                # one contiguous DMA per 1 MB half
                halves = []
                for hf in (0, 1):
                    wt = wpool.tile([128, NCI, 512], bf,
                                    tag=f"{tagpfx}h{hf}",
                                    name=f"{name}h{hf}")
                    engine.dma_start(out=wt, in_=w_d[wn, hf][:, :, :])
                    halves.append(wt)
                return [[halves[0][:, ci, :], halves[1][:, ci, :]]
                        for ci in range(NCI)]

            def proj_qk(w, x, b_sb, outT, psA, shpool, filler=None):
                """outT[:, co, :] = W @ x^T + b, then partial rotary.

                Groups of 2 co so accumulating + draining PSUM tiles
                double-buffer; rotary swap DMAs batched per group on the
                sync ring (the ACT queue stays pure compute).
                """
                for g in range(4):
                    c0 = 2 * g
                    pss = [psA.tile([128, 1024], f32, tag="ps",
                                    name=f"ps{c0 + k}") for k in (0, 1)]
                    for ci in range(NCI):
                        for k in (0, 1):
                            co = c0 + k
                            lhs = w[ci][co // 4][
                                :, (co % 4) * 128:(co % 4 + 1) * 128]
                            for (lo, hi) in (R0, R1):
                                nc.tensor.matmul(
                                    pss[k][:, lo:hi], lhs, x[:, ci, lo:hi],
                                    start=(ci == 0), stop=(ci == NCI - 1))
                        if filler is not None:
                            filler(g, ci)
                    for k in (0, 1):
                        nc.scalar.activation(
                            out=outT[:, c0 + k, :], in_=pss[k][:, 0:TP],
                            func=Ident, bias=b_sb[:, c0 + k:c0 + k + 1],
                            scale=1.0)
                    sh = shpool.tile([128, 2, TP], bf, tag="sh",
                                     name=f"sh{c0}")
                    nc.sync.dma_start(
                        out=sh[32:64, :, :], in_=outT[32:64, c0:c0 + 2, :])
                    for s in (0, 64):
                        nc.sync.dma_start(
                            out=sh[s:s + 16, :, :],
                            in_=outT[s + 16:s + 32, c0:c0 + 2, :])
                        nc.sync.dma_start(
                            out=sh[s + 16:s + 32, :, :],
                            in_=outT[s:s + 16, c0:c0 + 2, :])
                    for k in (0, 1):
                        co = c0 + k
                        nc.vector.tensor_mul(
                            sh[0:96, k, :], sh[0:96, k, :], sin_sb[0:96, :])
                        nc.vector.tensor_mul(
                            outT[:, co, :], outT[:, co, :], cos_sb)
                        nc.vector.tensor_add(
                            outT[0:96, co, :], outT[0:96, co, :],
                            sh[0:96, k, :])

            # ---- phase 1: Q/K projections ----
            with (
                tc.tile_pool(name="xkv", bufs=1) as xkp,
                tc.tile_pool(name="shpool", bufs=2) as shpool,
            ):
                xkv = xkp.tile([128, NCI, TP], bf, tag="xkv")
                bv_sb = xkp.tile([128, C], f32, tag="bv")
                with (
                    tc.tile_pool(name="psA", bufs=3, space="PSUM") as psA,
                    tc.tile_pool(name="psW", bufs=1, space="PSUM") as psW,
                    tc.tile_pool(name="xq", bufs=1) as xqp,
                ):
                    # PE warmup: keep the activity monitor busy while the
                    # first weight/x DMAs land, so matmuls run at 2.4 GHz.
                    pw = psW.tile([128, 512], f32, tag="pw")
                    for _ in range(20):
                        nc.tensor.matmul(pw[:, 0:128], wz, wz,
                                         start=True, stop=True)

                    def warm_filler(g, ci):
                        # filler matmuls woven into early Q-proj emission:
                        # they run whenever the PE would otherwise idle on
                        # a DMA wait, keeping the HAM clock at 8/8.
                        n = 2 if g == 0 else 0
                        for _ in range(n):
                            nc.tensor.matmul(pw[:, 0:128], wz, wz,
                                             start=True, stop=True)

                    xq = xqp.tile([128, NCI, TP], bf, tag="xq")
                    # sync ring: wq h0, xq, wq h1 (each one big DMA);
                    # scalar ring: wk h0 (behind the small consts);
                    # gpsimd ring: xkv, wk h1, bv
                    wqh0 = wpool.tile([128, NCI, 512], bf, tag="wAh0",
                                      name="wqh0")
                    nc.sync.dma_start(out=wqh0, in_=w_d["wq", 0][:, :, :])
                    nc.sync.dma_start(out=xq, in_=xqT_d[:, :, :])
                    wqh1 = wpool.tile([128, NCI, 512], bf, tag="wAh1",
                                      name="wqh1")
                    nc.sync.dma_start(out=wqh1, in_=w_d["wq", 1][:, :, :])
                    wq = [[wqh0[:, ci, :], wqh1[:, ci, :]]
                          for ci in range(NCI)]
                    wkh0 = wpool.tile([128, NCI, 512], bf, tag="wBh0",
                                      name="wkh0")
                    nc.scalar.dma_start(out=wkh0, in_=w_d["wk", 0][:, :, :])
                    nc.gpsimd.dma_start(out=xkv, in_=xkvT_d[:, :, :])
                    wkh1 = wpool.tile([128, NCI, 512], bf, tag="wBh1",
                                      name="wkh1")
                    nc.gpsimd.dma_start(out=wkh1, in_=w_d["wk", 1][:, :, :])
                    wk = [[wkh0[:, ci, :], wkh1[:, ci, :]]
                          for ci in range(NCI)]
                    nc.gpsimd.dma_start(
                        out=bv_sb, in_=bv_d[0:1, :].broadcast_to((128, C)))

                    proj_qk(wq, xq, bq_sb, qT, psA, shpool,
                            filler=warm_filler)
                    # wv prefetch on sync ring (lands during Q/K compute)
                    wv = load_w("wv", nc.sync, "wV", "wv")
                    proj_qk(wk, xkv, bk_sb, kT, psA, shpool)

                # ---- phase 2+3 ----
                with tc.tile_pool(name="ypool", bufs=1) as ypool:
                  yT = ypool.tile([128, NCI, TP], bf, tag="yT")
                  with (
                    tc.tile_pool(name="ptp", bufs=2) as ptpool,
                    tc.tile_pool(name="qzp", bufs=1) as qzpool,
                    tc.tile_pool(name="psS", bufs=2, space="PSUM") as psS,
                    tc.tile_pool(name="stgp", bufs=2) as stgpool,
                    tc.tile_pool(name="rdp", bufs=1) as rdpool,
                    tc.tile_pool(name="ycp", bufs=2) as ycpool,
                  ):
                    # wp reuses wq's tags/buffers (dead after Q-proj)
                    wp = load_w("wp", nc.sync, "wA", "wp")
                    bp_sb = ypool.tile([128, C], f32, tag="bp")
                    nc.gpsimd.dma_start(
                        out=bp_sb, in_=bp_d[0:1, :].broadcast_to((128, C)))


                    qza = [qzpool.tile([128, TP], bf, tag=f"qza{i}",
                                       name=f"qza{i}")
                           for i in (0, 1)]
                    qzb = [qzpool.tile([128, TP], bf, tag=f"qzb{i}",
                                       name=f"qzb{i}")
                           for i in (0, 1)]
                    for i in (0, 1):
                        nc.vector.memset(qza[i][64:128, :], 0.0)
                        nc.vector.memset(qzb[i][0:64, :], 0.0)

                    def emit_qz(j):
                        nc.vector.tensor_copy(
                            qza[j % 2][0:64, :], qT[0:64, j, :])
                        nc.vector.tensor_copy(
                            qzb[j % 2][64:128, :], qT[64:128, j, :])

                    pts = {h: {} for h in range(H)}

                    def emit_st(h, nk):
                        j, par = h // 2, h % 2
                        qzt = qza[j % 2] if par == 0 else qzb[j % 2]
                        qlo, qhi, moff = R0SUB[nk]
                        ps = psS.tile([128, 1024], f32, tag="st",
                                      name=f"st{h}_{nk}")
                        nc.tensor.matmul(
                            ps[:, qlo:qhi],
                            kT[:, j, nk * 128:(nk + 1) * 128],
                            qzt[:, qlo:qhi], start=True, stop=True)
                        nc.tensor.matmul(
                            ps[:, 512:770],
                            kT[:, j, nk * 128:(nk + 1) * 128],
                            qzt[:, 512:770], start=True, stop=True)
                        pt = ptpool.tile([128, 772], bf, tag=f"pt{par}{nk}",
                                         name=f"pt{h}_{nk}")
                        nc.scalar.activation(
                            out=pt[:, qlo:770], in_=ps[:, qlo:770],
                            func=Exp, scale=0.125)
                        if moff is not None:
                            nc.gpsimd.tensor_mul(
                                pt[:, moff:moff + 128],
                                pt[:, moff:moff + 128], m0_sb)
                        pts[h][nk] = pt

                    def emit_st_tail(h):
                        j, par = h // 2, h % 2
                        qzt = qza[j % 2] if par == 0 else qzb[j % 2]
                        ps = psS.tile([128, 1024], f32, tag="st",
                                      name=f"st{h}_6")
                        nc.tensor.matmul(
                            ps[0:1, 0:258], kT[:, j, 768:769],
                            qzt[:, 512:770], start=True, stop=True)
                        pt6 = ptpool.tile([1, 772], bf, tag=f"pt{par}6",
                                          name=f"pt6_{h}")
                        nc.scalar.activation(
                            out=pt6[0:1, 513:770], in_=ps[0:1, 1:258],
                            func=Exp, scale=0.125)
                        nc.vector.tensor_copy(
                            pt6[0:1, 512:513], zbf[0:1, 0:1])
                        pts[h][6] = pt6

                    def emit_pv(h, nk, o):
                        vs = slice(h * (HD + 1), h * (HD + 1) + 128)
                        qlo, qhi, _ = R0SUB[nk]
                        nc.tensor.matmul(
                            o[:, qlo:qhi], vaug[:, nk, vs],
                            pts[h][nk][:, qlo:qhi],
                            start=(nk == 0), stop=False)
                        nc.tensor.matmul(
                            o[:, 512:770], vaug[:, nk, vs],
                            pts[h][nk][:, 512:770],
                            start=(nk == 0), stop=False)

                    def emit_pv_tail(h, o):
                        vs = slice(h * (HD + 1), h * (HD + 1) + 128)
                        nc.tensor.matmul(
                            o[:, 512:770], vaug[0:1, 6, vs],
                            pts[h][6][0:1, 512:770],
                            start=False, stop=True)

                    def emit_div_a(j, oa, ob):
                        # copy numerators+denominator rows out of PSUM in
                        # one wide DVE op per head — this alone releases
                        # psO for the next pair; everything downstream is
                        # off the PE's critical path.
                        yca = ycpool.tile([65, TP], bf, tag="yca",
                                          name=f"yca{j}")
                        ycb = ycpool.tile([65, TP], bf, tag="ycb",
                                          name=f"ycb{j}")
                        nc.vector.tensor_copy(yca[0:65, :], oa[0:65, 0:770])
                        nc.vector.tensor_copy(ycb[0:65, :], ob[0:65, 0:770])
                        return yca, ycb

                    def emit_div_b(j, yca, ycb):
                        # ones-row K=1 matmul broadcasts the raw denominator
                        # across 64 partitions; reciprocal on 64 DVE lanes
                        # straight out of PSUM; then the two division muls.
                        pba = psS.tile([128, 1024], f32, tag="st",
                                       name=f"pba{j}")
                        pbb = psS.tile([128, 1024], f32, tag="st",
                                       name=f"pbb{j}")
                        for (lo, hi) in (R0, R1):
                            nc.tensor.matmul(pba[0:64, lo:hi],
                                             ones128[64:65, 0:64],
                                             yca[64:65, lo:hi],
                                             start=True, stop=True)
                            nc.tensor.matmul(pbb[0:64, lo:hi],
                                             ones128[64:65, 0:64],
                                             ycb[64:65, lo:hi],
                                             start=True, stop=True)
                        ra = rdpool.tile([64, TP], f32, tag="ra",
                                         name=f"ra{j}")
                        rb = rdpool.tile([64, TP], f32, tag="rb",
                                         name=f"rb{j}")
                        nc.vector.reciprocal_approx_fast(
                            out=ra, in_=pba[0:64, 0:770])
                        nc.vector.reciprocal_approx_fast(
                            out=rb, in_=pbb[0:64, 0:770])
                        nc.vector.tensor_mul(
                            yT[0:64, j, :], yca[0:64, :], ra)
                        nc.vector.tensor_mul(
                            yT[64:128, j, :], ycb[0:64, :], rb)

                    emit_qz(0)
                    emit_qz(1)

                    # ---- V projection, interleaved with pair 0/1 S^T+exp
                    # (ACT gets a head start while the PE does V) ----
                    pre_units = []
                    for h in (0, 1, 2, 3):
                        for nk in range(6):
                            pre_units.append((h, nk))
                        pre_units.append((h, 6))

                    def emit_vgroup(g, wv):
                        pss = {}
                        for tt in g:
                            pss[tt] = psV.tile([128, 1024], f32, tag="psv",
                                               name=f"psv{tt}")
                        for ci in range(NCI):
                            for tt in g:
                                tsz = 128 if tt < 6 else 1
                                lhs = xkv[:, ci, tt * 128:tt * 128 + tsz]
                                for hf in (0, 1):
                                    nc.tensor.matmul(
                                        pss[tt][:tsz,
                                                hf * 512:hf * 512 + 512],
                                        lhs, wv[ci][hf],
                                        start=(ci == 0),
                                        stop=(ci == NCI - 1))
                        for tt in g:
                            tsz = 128 if tt < 6 else 1
                            va = vaug[:tsz, tt, 0:H * (HD + 1)].rearrange(
                                "p (h e) -> p h e", e=HD + 1)
                            nc.vector.tensor_add(
                                va[:, :, 0:HD],
                                pss[tt][:tsz, :].rearrange(
                                    "p (h d) -> p h d", h=H),
                                bv_sb[:tsz, :].rearrange(
                                    "p (h d) -> p h d", h=H))
                            nc.vector.tensor_copy(
                                va[:, :, HD:HD + 1],
                                ones16[:tsz, :].unsqueeze(2))
                            nc.vector.tensor_copy(
                                vaug[:tsz, tt, H * (HD + 1):VW],
                                zbf[:tsz, 0:VW - H * (HD + 1)])

                    with tc.tile_pool(name="psV", bufs=2,
                                      space="PSUM") as psV:
                        vgroups = [(0, 1), (2, 3), (4, 5), (6,)]
                        ui = 0
                        for gi, g in enumerate(vgroups):
                            emit_vgroup(g, wv)
                            n_units = 7 * (gi + 1)
                            while ui < min(n_units, len(pre_units)):
                                h, nk = pre_units[ui]
                                if nk == 6:
                                    emit_st_tail(h)
                                else:
                                    emit_st(h, nk)
                                ui += 1
                        while ui < len(pre_units):
                            h, nk = pre_units[ui]
                            if nk == 6:
                                emit_st_tail(h)
                            else:
                                emit_st(h, nk)
                            ui += 1

                    # ---- attention pairs ----
                    with tc.tile_pool(name="psO", bufs=1,
                                      space="PSUM") as psO:
                        pending = None  # previous pair's deferred division
                        for j in range(NCI):
                            a, bh = 2 * j, 2 * j + 1
                            oa = psO.tile([128, 1024], f32, tag="ova",
                                          name=f"ov{a}")
                            ob = psO.tile([128, 1024], f32, tag="ovb",
                                          name=f"ov{bh}")
                            for nk in range(6):
                                # S^T one nk ahead of PV so the exp+mask
                                # chain has a full nk of slack
                                if j >= 2:
                                    if nk < 5:
                                        emit_st(a, nk + 1)
                                        emit_st(bh, nk + 1)
                                    else:
                                        emit_st_tail(a)
                                        emit_st_tail(bh)
                                if nk == 2 and pending is not None:
                                    emit_div_b(*pending)
                                    pending = None
                                emit_pv(a, nk, oa)
                                emit_pv(bh, nk, ob)
                            # cross-pair lookahead: next pair's first S^T
                            # before this pair's tails, so ACT never idles
                            # across the pair boundary
                            if j + 1 >= 2 and j + 1 < NCI:
                                emit_st(a + 2, 0)
                                emit_st(bh + 2, 0)
                            emit_pv_tail(a, oa)
                            emit_pv_tail(bh, ob)
                            yca, ycb = emit_div_a(j, oa, ob)
                            pending = (j, yca, ycb)
                            # lookahead qz AFTER this pair's S^T emissions
                            # (correct WAR/RAW) and after the psO-releasing
                            # copies (DVE queue priority)
                            if j + 2 < NCI:
                                emit_qz(j + 2)
                        emit_div_b(*pending)

                  # ---- phase 3: output projection ----
                  with (
                    tc.tile_pool(name="psF", bufs=4, space="PSUM") as psF,
                    tc.tile_pool(name="opool", bufs=3) as opool,
                  ):
                    for g in (range(0, 4), range(4, 7)):
                        pss = {}
                        for tt in g:
                            pss[tt] = psF.tile([128, 1024], f32, tag="pso",
                                               name=f"pso{tt}")
                        for ci in range(NCI):
                            for tt in g:
                                tsz = 128 if tt < 6 else 1
                                lhs = yT[:, ci, tt * 128:tt * 128 + tsz]
                                for hf in (0, 1):
                                    nc.tensor.matmul(
                                        pss[tt][:tsz,
                                                hf * 512:hf * 512 + 512],
                                        lhs, wp[ci][hf],
                                        start=(ci == 0),
                                        stop=(ci == NCI - 1))
                        for tt in g:
                            tsz = 128 if tt < 6 else 1
                            ot = opool.tile([128, 1024], f32, tag="ot",
                                            name="ot")
                            nc.vector.tensor_add(
                                ot[:tsz, :], pss[tt][:tsz, :],
                                bp_sb[:tsz, :])
                            eng = nc.sync if tt % 2 == 0 else nc.scalar
                            eng.dma_start(
                                out=out_d[tt * 128:tt * 128 + tsz, :],
                                in_=ot[:tsz, :])

    nc.compile()
    return nc


def _host_prep(x_q, x_kv, rotary_pos_emb, Wq, bq, Wk, bk, Wv, bv, Wp, bp):
    import ml_dtypes
    f = np.float32
    bft = ml_dtypes.bfloat16
    x_q = np.asarray(x_q, f)
    x_kv = np.asarray(x_kv, f)
    freqs = np.asarray(rotary_pos_emb, f)

    # Even/odd pair-split permutation of the first 32 dims of each head, so
    # rotate_half becomes a 16-partition block swap on chip.
    perm = np.arange(C)
    for h in range(H):
        b0 = h * HD
        blk = np.empty(HD, np.int64)
        blk[0:16] = b0 + np.arange(0, 32, 2)
        blk[16:32] = b0 + np.arange(1, 32, 2)
        blk[32:64] = b0 + np.arange(32, 64)
        perm[b0:b0 + HD] = blk

    def wT(W, p=None):
        # -> two [128, NCI, 512] halves, contiguous per partition row
        W = np.asarray(W, f)
        if p is not None:
            W = W[p, :]
        Wt = W.T.reshape(NCI, 128, C).transpose(1, 0, 2)  # [128, NCI, C]
        return (np.ascontiguousarray(Wt[:, :, 0:512]).astype(bft),
                np.ascontiguousarray(Wt[:, :, 512:1024]).astype(bft))

    cosE = np.cos(freqs[:, 0::2]).T  # [16, T]
    cosO = np.cos(freqs[:, 1::2]).T
    sinE = -np.sin(freqs[:, 0::2]).T
    sinO = np.sin(freqs[:, 1::2]).T
    cosP = np.ones((128, TP), f)
    sinP = np.zeros((128, TP), f)
    for s in (0, 64):
        cosP[s:s + 16, :T] = cosE
        cosP[s + 16:s + 32, :T] = cosO
        sinP[s:s + 16, :T] = sinE
        sinP[s + 16:s + 32, :T] = sinO

    p_idx = np.arange(128)[:, None]
    f_idx = np.arange(128)[None, :]
    m0 = (p_idx < f_idx).astype(f)

    bqp = np.asarray(bq, f)[perm]
    bkp = np.asarray(bk, f)[perm]
    shared = {
        "bq2": np.ascontiguousarray(bqp.reshape(NCO, 128).T),
        "bk2": np.ascontiguousarray(bkp.reshape(NCO, 128).T),
        "bv1": np.asarray(bv, f).reshape(1, C).copy(),
        "bp1": np.asarray(bp, f).reshape(1, C).copy(),
        "cosP": np.ascontiguousarray(cosP).astype(bft),
        "sinP": np.ascontiguousarray(sinP).astype(bft),
        "m0": np.ascontiguousarray(m0).astype(bft),
    }
    for wn, W, p in (("wq", Wq, perm), ("wk", Wk, perm),
                     ("wv", Wv, None), ("wp", Wp, None)):
        h0, h1 = wT(W, p)
        shared[f"{wn}T0"] = h0
        shared[f"{wn}T1"] = h1

    def padT(xt):
        # [C, T] -> [128, NCI, TP] (partition-contiguous)
        out = np.zeros((C, TP), f)
        out[:, :T] = xt
        out = out.reshape(NCI, 128, TP).transpose(1, 0, 2)
        return np.ascontiguousarray(out).astype(bft)

    in_maps = []
    for b in range(B):
        m = dict(shared)
        m["xqT"] = padT(x_q[b].T)
        m["xkvT"] = padT(x_kv[b].T)
        in_maps.append(m)
    return in_maps


def kernel(x_q, x_kv, rotary_pos_emb, Wq, bq, Wk, bk, Wv, bv, Wp, bp):
    from concourse.bass_utils import run_bass_kernel_spmd

    if "nc" not in _CACHE:
        _CACHE["nc"] = _build_program()
    nc = _CACHE["nc"]

    in_maps = _host_prep(x_q, x_kv, rotary_pos_emb,
                         Wq, bq, Wk, bk, Wv, bv, Wp, bp)
    trace = os.environ.get("BTK_TRACE", "0") == "1"
    res = run_bass_kernel_spmd(
        nc, in_maps, core_ids=list(range(B)), trace=trace)
    _CACHE["last_result"] = res
    return np.stack([r["out"] for r in res.results], axis=0)


# revision 51
# speedup vs baseline: 1.0278x; 1.0081x over previous
"""Trainium2 Bass kernel for CausalCrossAttention (B=8, T=769, C=1024, H=16).

Sharding: data-parallel over batch B=8 across the 8 NeuronCores (one batch
element per core, SPMD).

v2 (vs the fp32r baseline): all matmul operands in bf16 (fp32 PSUM accum),
DMA ring hygiene (bulk loads on sync+gpsimd rings, dependent SBUF swaps on
the scalar ring), per-projection weight tags so the next matrix prefetches
during the current projection, PE warmup matmuls under the initial DMAs,
V-projection overlapped with the first two head-pairs' S^T+exp (ACT head
start), per-nk interleaved attention (S^T a/b, exp, PV a/b) with an exact
8-bank PSUM budget, and an on-chip softmax denominator path
(DVE reciprocal + GpSimd partition_broadcast — no DRAM bounce).
"""

import os

import numpy as np

B, T, C = 8, 769, 1024
H, HD, L = 16, 64, 32
COND = 256
NCI = 8  # 1024 / 128 contraction tiles
NCO = 8
NTT = 7  # t tiles: 6 full + 1 single row
TP = 770  # streamed T padded to even
R0 = (0, 512)
R1 = (512, 770)
VW = H * (HD + 1) + 63  # V_aug free width

_CACHE = {}


def _build_program():
    import concourse.mybir as mybir
    import concourse.tile as tile
    from concourse import bacc

    f32 = mybir.dt.float32
    f32r = mybir.dt.float32r
    bf = mybir.dt.bfloat16
    Exp = mybir.ActivationFunctionType.Exp
    Ident = mybir.ActivationFunctionType.Identity

    nc = bacc.Bacc("TRN2", target_bir_lowering=False)

    # all bulk inputs pre-arranged on host to [128 partitions, ...] so every
    # DMA is one fully-contiguous-per-partition transfer at line rate
    xqT_d = nc.dram_tensor("xqT", [128, NCI, TP], bf, kind="ExternalInput")
    xkvT_d = nc.dram_tensor("xkvT", [128, NCI, TP], bf,
                            kind="ExternalInput")
    w_d = {}
    for wn in ("wq", "wk", "wv", "wp"):
        for hf in (0, 1):
            w_d[wn, hf] = nc.dram_tensor(f"{wn}T{hf}", [128, NCI, 512], bf,
                                         kind="ExternalInput")
    bq_d = nc.dram_tensor("bq2", [128, NCO], f32, kind="ExternalInput")
    bk_d = nc.dram_tensor("bk2", [128, NCO], f32, kind="ExternalInput")
    bv_d = nc.dram_tensor("bv1", [1, C], f32, kind="ExternalInput")
    bp_d = nc.dram_tensor("bp1", [1, C], f32, kind="ExternalInput")
    cos_d = nc.dram_tensor("cosP", [128, TP], bf, kind="ExternalInput")
    sin_d = nc.dram_tensor("sinP", [128, TP], bf, kind="ExternalInput")
    m0_d = nc.dram_tensor("m0", [128, 128], bf, kind="ExternalInput")
    out_d = nc.dram_tensor("out", [T, C], f32, kind="ExternalOutput")

    # Per-(kv-tile) q ranges in the 0:512 block + mask offset.
    R0SUB = {0: (0, 512, None), 1: (0, 512, None), 2: (0, 512, 0),
             3: (128, 512, 128), 4: (256, 512, 256), 5: (384, 512, 384)}

    with tile.TileContext(nc) as tc:
        with (
            tc.tile_pool(name="consts", bufs=1) as consts,
            tc.tile_pool(name="wpool", bufs=1) as wpool,
            tc.tile_pool(name="qkpool", bufs=1) as qkpool,
            tc.tile_pool(name="vpool", bufs=1) as vpool,
        ):
            cos_sb = consts.tile([128, TP], bf, tag="cos")
            sin_sb = consts.tile([128, TP], bf, tag="sin")
            m0_sb = consts.tile([128, 128], bf, tag="m0")
            bq_sb = consts.tile([128, NCO], f32, tag="bq")
            bk_sb = consts.tile([128, NCO], f32, tag="bk")
            ones16 = consts.tile([128, 16], f32, tag="ones16")
            nc.vector.memset(ones16, 1.0)
            zbf = consts.tile([128, TP], bf, tag="zbf")
            nc.vector.memset(zbf, 0.0)
            wz = consts.tile([128, 128], bf, tag="wz")
            nc.vector.memset(wz, 0.0)
            ones128 = consts.tile([128, 128], bf, tag="ones128")
            nc.vector.memset(ones128, 1.0)


            qT = qkpool.tile([128, NCI, TP], bf, tag="qT")
            kT = qkpool.tile([128, NCI, TP], bf, tag="kT")
            vaug = vpool.tile([128, NTT, VW], bf, tag="vaug")

            def load_w(wn, engine, tagpfx, name):
                # one contiguous DMA per 1 MB half
                halves = []
                for hf in (0, 1):
                    wt = wpool.tile([128, NCI, 512], bf,
                                    tag=f"{tagpfx}h{hf}",
                                    name=f"{name}h{hf}")
                    engine.dma_start(out=wt, in_=w_d[wn, hf][:, :, :])
                    halves.append(wt)
                return [[halves[0][:, ci, :], halves[1][:, ci, :]]
                        for ci in range(NCI)]

            def proj_qk(w, x, b_sb, outT, psA, shpool, filler=None):
                """outT[:, co, :] = W @ x^T + b, then partial rotary.

                Groups of 2 co so accumulating + draining PSUM tiles
                double-buffer; rotary swap DMAs batched per group on the
                sync ring (the ACT queue stays pure compute).
                """
                for g in range(4):
                    c0 = 2 * g
                    pss = [psA.tile([128, 1024], f32, tag="ps",
                                    name=f"ps{c0 + k}") for k in (0, 1)]
                    for ci in range(NCI):
                        for k in (0, 1):
                            co = c0 + k
                            lhs = w[ci][co // 4][
                                :, (co % 4) * 128:(co % 4 + 1) * 128]
                            for (lo, hi) in (R0, R1):
                                nc.tensor.matmul(
                                    pss[k][:, lo:hi], lhs, x[:, ci, lo:hi],
                                    start=(ci == 0), stop=(ci == NCI - 1))
                        if filler is not None:
                            filler(g, ci)
                    for k in (0, 1):
                        nc.scalar.activation(
                            out=outT[:, c0 + k, :], in_=pss[k][:, 0:TP],
                            func=Ident, bias=b_sb[:, c0 + k:c0 + k + 1],
                            scale=1.0)
                    sh = shpool.tile([128, 2, TP], bf, tag="sh",
                                     name=f"sh{c0}")
                    nc.sync.dma_start(
                        out=sh[32:64, :, :], in_=outT[32:64, c0:c0 + 2, :])
                    for s in (0, 64):
                        nc.sync.dma_start(
                            out=sh[s:s + 16, :, :],
                            in_=outT[s + 16:s + 32, c0:c0 + 2, :])
                        nc.sync.dma_start(
                            out=sh[s + 16:s + 32, :, :],
                            in_=outT[s:s + 16, c0:c0 + 2, :])
                    for k in (0, 1):
                        co = c0 + k
                        nc.vector.tensor_mul(
                            sh[0:96, k, :], sh[0:96, k, :], sin_sb[0:96, :])
                        nc.vector.tensor_mul(
                            outT[:, co, :], outT[:, co, :], cos_sb)
                        nc.vector.tensor_add(
                            outT[0:96, co, :], outT[0:96, co, :],
                            sh[0:96, k, :])

            # ---- phase 1: Q/K projections ----
            with (
                tc.tile_pool(name="xkv", bufs=1) as xkp,
                tc.tile_pool(name="shpool", bufs=2) as shpool,
            ):
                xkv = xkp.tile([128, NCI, TP], bf, tag="xkv")
                bv_sb = xkp.tile([128, C], f32, tag="bv")
                with (
                    tc.tile_pool(name="psA", bufs=3, space="PSUM") as psA,
                    tc.tile_pool(name="psW", bufs=1, space="PSUM") as psW,
                    tc.tile_pool(name="xq", bufs=1) as xqp,
                ):
                    # PE warmup: keep the activity monitor busy while the
                    # first weight/x DMAs land, so matmuls run at 2.4 GHz.
                    pw = psW.tile([128, 512], f32, tag="pw")
                    for _ in range(28):
                        nc.tensor.matmul(pw[:, 0:128], wz, wz,
                                         start=True, stop=True)

                    def warm_filler(g, ci):
                        # filler matmuls woven into early Q-proj emission:
                        # they run whenever the PE would otherwise idle on
                        # a DMA wait, keeping the HAM clock at 8/8.
                        n = 2 if g == 0 else 0
                        for _ in range(n):
                            nc.tensor.matmul(pw[:, 0:128], wz, wz,
                                             start=True, stop=True)

                    xq = xqp.tile([128, NCI, TP], bf, tag="xq")
                    # sync ring: wq h0, xq, wq h1 (each one big DMA);
                    # scalar ring: wk h0 (behind the small consts);
                    # gpsimd ring: xkv, wk h1, bv
                    wqh0 = wpool.tile([128, NCI, 512], bf, tag="wAh0",
                                      name="wqh0")
                    nc.sync.dma_start(out=wqh0, in_=w_d["wq", 0][:, :, :])
                    nc.scalar.dma_start(out=xq, in_=xqT_d[:, :, :])
                    wqh1 = wpool.tile([128, NCI, 512], bf, tag="wAh1",
                                      name="wqh1")
                    nc.sync.dma_start(out=wqh1, in_=w_d["wq", 1][:, :, :])
                    wq = [[wqh0[:, ci, :], wqh1[:, ci, :]]
                          for ci in range(NCI)]
                    # small consts after the critical xq on the scalar ring
                    nc.scalar.dma_start(out=bq_sb, in_=bq_d[:, :])
                    nc.scalar.dma_start(out=cos_sb, in_=cos_d[:, :])
                    nc.scalar.dma_start(out=sin_sb, in_=sin_d[:, :])
                    nc.scalar.dma_start(out=bk_sb, in_=bk_d[:, :])
                    wkh0 = wpool.tile([128, NCI, 512], bf, tag="wBh0",
                                      name="wkh0")
                    nc.scalar.dma_start(out=wkh0, in_=w_d["wk", 0][:, :, :])
                    nc.scalar.dma_start(out=m0_sb, in_=m0_d[:, :])
                    nc.gpsimd.dma_start(out=xkv, in_=xkvT_d[:, :, :])
                    wkh1 = wpool.tile([128, NCI, 512], bf, tag="wBh1",
                                      name="wkh1")
                    nc.gpsimd.dma_start(out=wkh1, in_=w_d["wk", 1][:, :, :])
                    wk = [[wkh0[:, ci, :], wkh1[:, ci, :]]
                          for ci in range(NCI)]
                    nc.gpsimd.dma_start(
                        out=bv_sb, in_=bv_d[0:1, :].broadcast_to((128, C)))

                    proj_qk(wq, xq, bq_sb, qT, psA, shpool,
                            filler=warm_filler)
                    # wv prefetch on sync ring (lands during Q/K compute)
                    wv = load_w("wv", nc.sync, "wV", "wv")
                    proj_qk(wk, xkv, bk_sb, kT, psA, shpool)

                # ---- phase 2+3 ----
                with tc.tile_pool(name="ypool", bufs=1) as ypool:
                  yT = ypool.tile([128, NCI, TP], bf, tag="yT")
                  with (
                    tc.tile_pool(name="ptp", bufs=2) as ptpool,
                    tc.tile_pool(name="qzp", bufs=1) as qzpool,
                    tc.tile_pool(name="psS", bufs=2, space="PSUM") as psS,
                    tc.tile_pool(name="stgp", bufs=2) as stgpool,
                    tc.tile_pool(name="rdp", bufs=1) as rdpool,
                    tc.tile_pool(name="ycp", bufs=2) as ycpool,
                  ):
                    # wp reuses wq's tags/buffers (dead after Q-proj)
                    wp = load_w("wp", nc.sync, "wA", "wp")
                    bp_sb = ypool.tile([128, C], f32, tag="bp")
                    nc.gpsimd.dma_start(
                        out=bp_sb, in_=bp_d[0:1, :].broadcast_to((128, C)))


                    qza = [qzpool.tile([128, TP], bf, tag=f"qza{i}",
                                       name=f"qza{i}")
                           for i in (0, 1)]
                    qzb = [qzpool.tile([128, TP], bf, tag=f"qzb{i}",
                                       name=f"qzb{i}")
                           for i in (0, 1)]
                    for i in (0, 1):
                        nc.vector.memset(qza[i][64:128, :], 0.0)
                        nc.vector.memset(qzb[i][0:64, :], 0.0)

                    def emit_qz(j):
                        nc.vector.tensor_copy(
                            qza[j % 2][0:64, :], qT[0:64, j, :])
                        nc.vector.tensor_copy(
                            qzb[j % 2][64:128, :], qT[64:128, j, :])

                    pts = {h: {} for h in range(H)}

                    def emit_st(h, nk):
                        j, par = h // 2, h % 2
                        qzt = qza[j % 2] if par == 0 else qzb[j % 2]
                        qlo, qhi, moff = R0SUB[nk]
                        ps = psS.tile([128, 1024], f32, tag="st",
                                      name=f"st{h}_{nk}")
                        nc.tensor.matmul(
                            ps[:, qlo:qhi],
                            kT[:, j, nk * 128:(nk + 1) * 128],
                            qzt[:, qlo:qhi], start=True, stop=True)
                        nc.tensor.matmul(
                            ps[:, 512:770],
                            kT[:, j, nk * 128:(nk + 1) * 128],
                            qzt[:, 512:770], start=True, stop=True)
                        pt = ptpool.tile([128, 772], bf, tag=f"pt{par}{nk}",
                                         name=f"pt{h}_{nk}")
                        nc.scalar.activation(
                            out=pt[:, qlo:770], in_=ps[:, qlo:770],
                            func=Exp, scale=0.125)
                        if moff is not None:
                            nc.gpsimd.tensor_mul(
                                pt[:, moff:moff + 128],
                                pt[:, moff:moff + 128], m0_sb)
                        pts[h][nk] = pt

                    def emit_st_tail(h):
                        j, par = h // 2, h % 2
                        qzt = qza[j % 2] if par == 0 else qzb[j % 2]
                        ps = psS.tile([128, 1024], f32, tag="st",
                                      name=f"st{h}_6")
                        nc.tensor.matmul(
                            ps[0:1, 0:258], kT[:, j, 768:769],
                            qzt[:, 512:770], start=True, stop=True)
                        pt6 = ptpool.tile([1, 772], bf, tag=f"pt{par}6",
                                          name=f"pt6_{h}")
                        nc.scalar.activation(
                            out=pt6[0:1, 513:770], in_=ps[0:1, 1:258],
                            func=Exp, scale=0.125)
                        nc.vector.tensor_copy(
                            pt6[0:1, 512:513], zbf[0:1, 0:1])
                        pts[h][6] = pt6

                    def emit_pv(h, nk, o):
                        vs = slice(h * (HD + 1), h * (HD + 1) + 128)
                        qlo, qhi, _ = R0SUB[nk]
                        nc.tensor.matmul(
                            o[:, qlo:qhi], vaug[:, nk, vs],
                            pts[h][nk][:, qlo:qhi],
                            start=(nk == 0), stop=False)
                        nc.tensor.matmul(
                            o[:, 512:770], vaug[:, nk, vs],
                            pts[h][nk][:, 512:770],
                            start=(nk == 0), stop=False)

                    def emit_pv_tail(h, o):
                        vs = slice(h * (HD + 1), h * (HD + 1) + 128)
                        nc.tensor.matmul(
                            o[:, 512:770], vaug[0:1, 6, vs],
                            pts[h][6][0:1, 512:770],
                            start=False, stop=True)

                    def emit_div_a(j, oa, ob):
                        # copy numerators+denominator rows out of PSUM in
                        # one wide DVE op per head — this alone releases
                        # psO for the next pair; everything downstream is
                        # off the PE's critical path.
                        yca = ycpool.tile([65, TP], bf, tag="yca",
                                          name=f"yca{j}")
                        ycb = ycpool.tile([65, TP], bf, tag="ycb",
                                          name=f"ycb{j}")
                        nc.vector.tensor_copy(yca[0:65, :], oa[0:65, 0:770])
                        nc.vector.tensor_copy(ycb[0:65, :], ob[0:65, 0:770])
                        return yca, ycb

                    def emit_div_b(j, yca, ycb):
                        # ones-row K=1 matmul broadcasts the raw denominator
                        # across 64 partitions; reciprocal on 64 DVE lanes
                        # straight out of PSUM; then the two division muls.
                        pba = psS.tile([128, 1024], f32, tag="st",
                                       name=f"pba{j}")
                        pbb = psS.tile([128, 1024], f32, tag="st",
                                       name=f"pbb{j}")
                        for (lo, hi) in (R0, R1):
                            nc.tensor.matmul(pba[0:64, lo:hi],
                                             ones128[64:65, 0:64],
                                             yca[64:65, lo:hi],
                                             start=True, stop=True)
                            nc.tensor.matmul(pbb[0:64, lo:hi],
                                             ones128[64:65, 0:64],
                                             ycb[64:65, lo:hi],
                                             start=True, stop=True)
                        ra = rdpool.tile([64, TP], f32, tag="ra",
                                         name=f"ra{j}")
                        rb = rdpool.tile([64, TP], f32, tag="rb",
                                         name=f"rb{j}")
                        nc.vector.reciprocal_approx_fast(
                            out=ra, in_=pba[0:64, 0:770])
                        nc.vector.reciprocal_approx_fast(
                            out=rb, in_=pbb[0:64, 0:770])
                        nc.vector.tensor_mul(
                            yT[0:64, j, :], yca[0:64, :], ra)
                        nc.vector.tensor_mul(
                            yT[64:128, j, :], ycb[0:64, :], rb)

                    emit_qz(0)
                    emit_qz(1)

                    # ---- V projection, interleaved with pair 0/1 S^T+exp
                    # (ACT gets a head start while the PE does V) ----
                    pre_units = []
                    for h in (0, 1, 2, 3):
                        for nk in range(6):
                            pre_units.append((h, nk))
                        pre_units.append((h, 6))

                    def emit_vgroup(g, wv):
                        pss = {}
                        for tt in g:
                            pss[tt] = psV.tile([128, 1024], f32, tag="psv",
                                               name=f"psv{tt}")
                        for ci in range(NCI):
                            for tt in g:
                                tsz = 128 if tt < 6 else 1
                                lhs = xkv[:, ci, tt * 128:tt * 128 + tsz]
                                for hf in (0, 1):
                                    nc.tensor.matmul(
                                        pss[tt][:tsz,
                                                hf * 512:hf * 512 + 512],
                                        lhs, wv[ci][hf],
                                        start=(ci == 0),
                                        stop=(ci == NCI - 1))
                        for tt in g:
                            tsz = 128 if tt < 6 else 1
                            va = vaug[:tsz, tt, 0:H * (HD + 1)].rearrange(
                                "p (h e) -> p h e", e=HD + 1)
                            nc.vector.tensor_add(
                                va[:, :, 0:HD],
                                pss[tt][:tsz, :].rearrange(
                                    "p (h d) -> p h d", h=H),
                                bv_sb[:tsz, :].rearrange(
                                    "p (h d) -> p h d", h=H))
                            nc.vector.tensor_copy(
                                va[:, :, HD:HD + 1],
                                ones16[:tsz, :].unsqueeze(2))
                            nc.vector.tensor_copy(
                                vaug[:tsz, tt, H * (HD + 1):VW],
                                zbf[:tsz, 0:VW - H * (HD + 1)])

                    with tc.tile_pool(name="psV", bufs=2,
                                      space="PSUM") as psV:
                        vgroups = [(0, 1), (2, 3), (4, 5), (6,)]
                        ui = 0
                        for gi, g in enumerate(vgroups):
                            emit_vgroup(g, wv)
                            n_units = 7 * (gi + 1)
                            while ui < min(n_units, len(pre_units)):
                                h, nk = pre_units[ui]
                                if nk == 6:
                                    emit_st_tail(h)
                                else:
                                    emit_st(h, nk)
                                ui += 1
                        while ui < len(pre_units):
                            h, nk = pre_units[ui]
                            if nk == 6:
                                emit_st_tail(h)
                            else:
                                emit_st(h, nk)
                            ui += 1

                    # ---- attention pairs ----
                    with tc.tile_pool(name="psO", bufs=1,
                                      space="PSUM") as psO:
                        pending = None  # previous pair's deferred division
                        for j in range(NCI):
                            a, bh = 2 * j, 2 * j + 1
                            oa = psO.tile([128, 1024], f32, tag="ova",
                                          name=f"ov{a}")
                            ob = psO.tile([128, 1024], f32, tag="ovb",
                                          name=f"ov{bh}")
                            for nk in range(6):
                                # S^T one nk ahead of PV so the exp+mask
                                # chain has a full nk of slack
                                if j >= 2:
                                    if nk < 5:
                                        emit_st(a, nk + 1)
                                        emit_st(bh, nk + 1)
                                    else:
                                        emit_st_tail(a)
                                        emit_st_tail(bh)
                                if nk == 2 and pending is not None:
                                    emit_div_b(*pending)
                                    pending = None
                                emit_pv(a, nk, oa)
                                emit_pv(bh, nk, ob)
                            # cross-pair lookahead: next pair's first S^T
                            # before this pair's tails, so ACT never idles
                            # across the pair boundary
                            if j + 1 >= 2 and j + 1 < NCI:
                                emit_st(a + 2, 0)
                                emit_st(bh + 2, 0)
                            emit_pv_tail(a, oa)
                            emit_pv_tail(bh, ob)
                            yca, ycb = emit_div_a(j, oa, ob)
                            pending = (j, yca, ycb)
                            # lookahead qz AFTER this pair's S^T emissions
                            # (correct WAR/RAW) and after the psO-releasing
                            # copies (DVE queue priority)
                            if j + 2 < NCI:
                                emit_qz(j + 2)
                        emit_div_b(*pending)

                  # ---- phase 3: output projection ----
                  with (
                    tc.tile_pool(name="psF", bufs=4, space="PSUM") as psF,
                    tc.tile_pool(name="opool", bufs=3) as opool,
                  ):
                    for g in (range(0, 4), range(4, 7)):
                        pss = {}
                        for tt in g:
                            pss[tt] = psF.tile([128, 1024], f32, tag="pso",
                                               name=f"pso{tt}")
                        for ci in range(NCI):
                            for tt in g:
                                tsz = 128 if tt < 6 else 1
                                lhs = yT[:, ci, tt * 128:tt * 128 + tsz]
                                for hf in (0, 1):
                                    nc.tensor.matmul(
                                        pss[tt][:tsz,
                                                hf * 512:hf * 512 + 512],
                                        lhs, wp[ci][hf],
                                        start=(ci == 0),
                                        stop=(ci == NCI - 1))
                        for tt in g:
                            tsz = 128 if tt < 6 else 1
                            ot = opool.tile([128, 1024], f32, tag="ot",
                                            name="ot")
                            nc.vector.tensor_add(
                                ot[:tsz, :], pss[tt][:tsz, :],
                                bp_sb[:tsz, :])
                            eng = nc.sync if tt % 2 == 0 else nc.scalar
                            eng.dma_start(
                                out=out_d[tt * 128:tt * 128 + tsz, :],
                                in_=ot[:tsz, :])

    nc.compile()
    return nc


def _host_prep(x_q, x_kv, rotary_pos_emb, Wq, bq, Wk, bk, Wv, bv, Wp, bp):
    import ml_dtypes
    f = np.float32
    bft = ml_dtypes.bfloat16
    x_q = np.asarray(x_q, f)
    x_kv = np.asarray(x_kv, f)
    freqs = np.asarray(rotary_pos_emb, f)

    # Even/odd pair-split permutation of the first 32 dims of each head, so
    # rotate_half becomes a 16-partition block swap on chip.
    perm = np.arange(C)
    for h in range(H):
        b0 = h * HD
        blk = np.empty(HD, np.int64)
        blk[0:16] = b0 + np.arange(0, 32, 2)
        blk[16:32] = b0 + np.arange(1, 32, 2)
        blk[32:64] = b0 + np.arange(32, 64)
        perm[b0:b0 + HD] = blk

    def wT(W, p=None):
        # -> two [128, NCI, 512] halves, contiguous per partition row
        W = np.asarray(W, f)
        if p is not None:
            W = W[p, :]
        Wt = W.T.reshape(NCI, 128, C).transpose(1, 0, 2)  # [128, NCI, C]
        return (np.ascontiguousarray(Wt[:, :, 0:512]).astype(bft),
                np.ascontiguousarray(Wt[:, :, 512:1024]).astype(bft))

    cosE = np.cos(freqs[:, 0::2]).T  # [16, T]
    cosO = np.cos(freqs[:, 1::2]).T
    sinE = -np.sin(freqs[:, 0::2]).T
    sinO = np.sin(freqs[:, 1::2]).T
    cosP = np.ones((128, TP), f)
    sinP = np.zeros((128, TP), f)
    for s in (0, 64):
        cosP[s:s + 16, :T] = cosE
        cosP[s + 16:s + 32, :T] = cosO
        sinP[s:s + 16, :T] = sinE
        sinP[s + 16:s + 32, :T] = sinO

    p_idx = np.arange(128)[:, None]
    f_idx = np.arange(128)[None, :]
    m0 = (p_idx < f_idx).astype(f)

    bqp = np.asarray(bq, f)[perm]
    bkp = np.asarray(bk, f)[perm]
    shared = {
        "bq2": np.ascontiguousarray(bqp.reshape(NCO, 128).T),
        "bk2": np.ascontiguousarray(bkp.reshape(NCO, 128).T),
        "bv1": np.asarray(bv, f).reshape(1, C).copy(),
        "bp1": np.asarray(bp, f).reshape(1, C).copy(),
        "cosP": np.ascontiguousarray(cosP).astype(bft),
        "sinP": np.ascontiguousarray(sinP).astype(bft),
        "m0": np.ascontiguousarray(m0).astype(bft),
    }
    for wn, W, p in (("wq", Wq, perm), ("wk", Wk, perm),
                     ("wv", Wv, None), ("wp", Wp, None)):
        h0, h1 = wT(W, p)
        shared[f"{wn}T0"] = h0
        shared[f"{wn}T1"] = h1

    def padT(xt):
        # [C, T] -> [128, NCI, TP] (partition-contiguous)
        out = np.zeros((C, TP), f)
        out[:, :T] = xt
        out = out.reshape(NCI, 128, TP).transpose(1, 0, 2)
        return np.ascontiguousarray(out).astype(bft)

    in_maps = []
    for b in range(B):
        m = dict(shared)
        m["xqT"] = padT(x_q[b].T)
        m["xkvT"] = padT(x_kv[b].T)
        in_maps.append(m)
    return in_maps


def kernel(x_q, x_kv, rotary_pos_emb, Wq, bq, Wk, bk, Wv, bv, Wp, bp):
    from concourse.bass_utils import run_bass_kernel_spmd

    if "nc" not in _CACHE:
        _CACHE["nc"] = _build_program()
    nc = _CACHE["nc"]

    in_maps = _host_prep(x_q, x_kv, rotary_pos_emb,
                         Wq, bq, Wk, bk, Wv, bv, Wp, bp)
    trace = os.environ.get("BTK_TRACE", "0") == "1"
    res = run_bass_kernel_spmd(
        nc, in_maps, core_ids=list(range(B)), trace=trace)
    _CACHE["last_result"] = res
    return np.stack([r["out"] for r in res.results], axis=0)


# revision 53
# speedup vs baseline: 1.0368x; 1.0088x over previous
"""Trainium2 Bass kernel for CausalCrossAttention (B=8, T=769, C=1024, H=16).

Sharding: data-parallel over batch B=8 across the 8 NeuronCores (one batch
element per core, SPMD).

v2 (vs the fp32r baseline): all matmul operands in bf16 (fp32 PSUM accum),
DMA ring hygiene (bulk loads on sync+gpsimd rings, dependent SBUF swaps on
the scalar ring), per-projection weight tags so the next matrix prefetches
during the current projection, PE warmup matmuls under the initial DMAs,
V-projection overlapped with the first two head-pairs' S^T+exp (ACT head
start), per-nk interleaved attention (S^T a/b, exp, PV a/b) with an exact
8-bank PSUM budget, and an on-chip softmax denominator path
(DVE reciprocal + GpSimd partition_broadcast — no DRAM bounce).
"""

import os

import numpy as np

B, T, C = 8, 769, 1024
H, HD, L = 16, 64, 32
COND = 256
NCI = 8  # 1024 / 128 contraction tiles
NCO = 8
NTT = 7  # t tiles: 6 full + 1 single row
TP = 770  # streamed T padded to even
R0 = (0, 512)
R1 = (512, 770)
VW = H * (HD + 1) + 63  # V_aug free width

_CACHE = {}


def _build_program():
    import concourse.mybir as mybir
    import concourse.tile as tile
    from concourse import bacc

    f32 = mybir.dt.float32
    f32r = mybir.dt.float32r
    bf = mybir.dt.bfloat16
    Exp = mybir.ActivationFunctionType.Exp
    Ident = mybir.ActivationFunctionType.Identity

    nc = bacc.Bacc("TRN2", target_bir_lowering=False)

    # all bulk inputs pre-arranged on host to [128 partitions, ...] so every
    # DMA is one fully-contiguous-per-partition transfer at line rate
    xqT_d = nc.dram_tensor("xqT", [128, NCI, TP], bf, kind="ExternalInput")
    xkvT_d = nc.dram_tensor("xkvT", [128, NCI, TP], bf,
                            kind="ExternalInput")
    w_d = {}
    for wn in ("wq", "wk", "wv", "wp"):
        for hf in (0, 1):
            w_d[wn, hf] = nc.dram_tensor(f"{wn}T{hf}", [128, NCI, 512], bf,
                                         kind="ExternalInput")
    bq_d = nc.dram_tensor("bq2", [128, NCO], f32, kind="ExternalInput")
    bk_d = nc.dram_tensor("bk2", [128, NCO], f32, kind="ExternalInput")
    bv_d = nc.dram_tensor("bv1", [1, C], f32, kind="ExternalInput")
    bp_d = nc.dram_tensor("bp1", [1, C], f32, kind="ExternalInput")
    cos_d = nc.dram_tensor("cosP", [128, TP], bf, kind="ExternalInput")
    sin_d = nc.dram_tensor("sinP", [128, TP], bf, kind="ExternalInput")
    m0_d = nc.dram_tensor("m0", [128, 128], bf, kind="ExternalInput")
    out_d = nc.dram_tensor("out", [T, C], f32, kind="ExternalOutput")

    # Per-(kv-tile) q ranges in the 0:512 block + mask offset.
    R0SUB = {0: (0, 512, None), 1: (0, 512, None), 2: (0, 512, 0),
             3: (128, 512, 128), 4: (256, 512, 256), 5: (384, 512, 384)}

    with tile.TileContext(nc) as tc:
        with (
            tc.tile_pool(name="consts", bufs=1) as consts,
            tc.tile_pool(name="wpool", bufs=1) as wpool,
            tc.tile_pool(name="qkpool", bufs=1) as qkpool,
            tc.tile_pool(name="vpool", bufs=1) as vpool,
        ):
            cos_sb = consts.tile([128, TP], bf, tag="cos")
            sin_sb = consts.tile([128, TP], bf, tag="sin")
            m0_sb = consts.tile([128, 128], bf, tag="m0")
            bq_sb = consts.tile([128, NCO], f32, tag="bq")
            bk_sb = consts.tile([128, NCO], f32, tag="bk")
            ones16 = consts.tile([128, 16], f32, tag="ones16")
            nc.vector.memset(ones16, 1.0)
            zbf = consts.tile([128, TP], bf, tag="zbf")
            nc.vector.memset(zbf, 0.0)
            wz = consts.tile([128, 128], bf, tag="wz")
            nc.vector.memset(wz, 0.0)
            ones128 = consts.tile([128, 128], bf, tag="ones128")
            nc.vector.memset(ones128, 1.0)


            qT = qkpool.tile([128, NCI, TP], bf, tag="qT")
            kT = qkpool.tile([128, NCI, TP], bf, tag="kT")
            vaug = vpool.tile([128, NTT, VW], bf, tag="vaug")

            def load_w(wn, engine, tagpfx, name):
                # one contiguous DMA per 1 MB half
                halves = []
                for hf in (0, 1):
                    wt = wpool.tile([128, NCI, 512], bf,
                                    tag=f"{tagpfx}h{hf}",
                                    name=f"{name}h{hf}")
                    engine.dma_start(out=wt, in_=w_d[wn, hf][:, :, :])
                    halves.append(wt)
                return [[halves[0][:, ci, :], halves[1][:, ci, :]]
                        for ci in range(NCI)]

            def proj_qk(w, x, b_sb, outT, psA, shpool, filler=None):
                """outT[:, co, :] = W @ x^T + b, then partial rotary.

                Groups of 2 co so accumulating + draining PSUM tiles
                double-buffer; rotary swap DMAs batched per group on the
                sync ring (the ACT queue stays pure compute).
                """
                for g in range(4):
                    c0 = 2 * g
                    pss = [psA.tile([128, 1024], f32, tag="ps",
                                    name=f"ps{c0 + k}") for k in (0, 1)]
                    for ci in range(NCI):
                        for k in (0, 1):
                            co = c0 + k
                            lhs = w[ci][co // 4][
                                :, (co % 4) * 128:(co % 4 + 1) * 128]
                            for (lo, hi) in (R0, R1):
                                nc.tensor.matmul(
                                    pss[k][:, lo:hi], lhs, x[:, ci, lo:hi],
                                    start=(ci == 0), stop=(ci == NCI - 1))
                        if filler is not None:
                            filler(g, ci)
                    for k in (0, 1):
                        nc.scalar.activation(
                            out=outT[:, c0 + k, :], in_=pss[k][:, 0:TP],
                            func=Ident, bias=b_sb[:, c0 + k:c0 + k + 1],
                            scale=1.0)
                    sh = shpool.tile([128, 2, TP], bf, tag="sh",
                                     name=f"sh{c0}")
                    nc.sync.dma_start(
                        out=sh[32:64, :, :], in_=outT[32:64, c0:c0 + 2, :])
                    for s in (0, 64):
                        nc.sync.dma_start(
                            out=sh[s:s + 16, :, :],
                            in_=outT[s + 16:s + 32, c0:c0 + 2, :])
                        nc.sync.dma_start(
                            out=sh[s + 16:s + 32, :, :],
                            in_=outT[s:s + 16, c0:c0 + 2, :])
                    for k in (0, 1):
                        co = c0 + k
                        nc.vector.tensor_mul(
                            sh[0:96, k, :], sh[0:96, k, :], sin_sb[0:96, :])
                        nc.vector.tensor_mul(
                            outT[:, co, :], outT[:, co, :], cos_sb)
                        nc.vector.tensor_add(
                            outT[0:96, co, :], outT[0:96, co, :],
                            sh[0:96, k, :])

            # ---- phase 1: Q/K projections ----
            with (
                tc.tile_pool(name="xkv", bufs=1) as xkp,
                tc.tile_pool(name="shpool", bufs=2) as shpool,
            ):
                xkv = xkp.tile([128, NCI, TP], bf, tag="xkv")
                bv_sb = xkp.tile([128, C], f32, tag="bv")
                with (
                    tc.tile_pool(name="psA", bufs=3, space="PSUM") as psA,
                    tc.tile_pool(name="psW", bufs=1, space="PSUM") as psW,
                    tc.tile_pool(name="xq", bufs=1) as xqp,
                ):
                    # PE warmup: keep the activity monitor busy while the
                    # first weight/x DMAs land, so matmuls run at 2.4 GHz.
                    pw = psW.tile([128, 512], f32, tag="pw")
                    for _ in range(8):
                        nc.tensor.matmul(pw[:, 0:128], wz, wz,
                                         start=True, stop=True)

                    def warm_filler(g, ci):
                        # filler matmuls woven into early Q-proj emission:
                        # they run whenever the PE would otherwise idle on
                        # a DMA wait, keeping the HAM clock at 8/8.
                        n = 9 if g == 0 else 0
                        for _ in range(n):
                            nc.tensor.matmul(pw[:, 0:128], wz, wz,
                                             start=True, stop=True)

                    xq = xqp.tile([128, NCI, TP], bf, tag="xq")
                    # sync ring: wq h0, xq, wq h1 (each one big DMA);
                    # scalar ring: wk h0 (behind the small consts);
                    # gpsimd ring: xkv, wk h1, bv
                    # per-ci chunks for the two tensors the first matmuls
                    # need, so compute starts while the bulk still loads
                    # (each chunk is partition-contiguous at line rate)
                    wqh0 = wpool.tile([128, NCI, 512], bf, tag="wAh0",
                                      name="wqh0")
                    for ci in range(NCI):
                        nc.sync.dma_start(out=wqh0[:, ci, :],
                                          in_=w_d["wq", 0][:, ci, :])
                        nc.scalar.dma_start(out=xq[:, ci, :],
                                            in_=xqT_d[:, ci, :])
                    wqh1 = wpool.tile([128, NCI, 512], bf, tag="wAh1",
                                      name="wqh1")
                    nc.sync.dma_start(out=wqh1, in_=w_d["wq", 1][:, :, :])
                    wq = [[wqh0[:, ci, :], wqh1[:, ci, :]]
                          for ci in range(NCI)]
                    # small consts after the critical xq on the scalar ring
                    nc.scalar.dma_start(out=bq_sb, in_=bq_d[:, :])
                    nc.scalar.dma_start(out=cos_sb, in_=cos_d[:, :])
                    nc.scalar.dma_start(out=sin_sb, in_=sin_d[:, :])
                    nc.scalar.dma_start(out=bk_sb, in_=bk_d[:, :])
                    wkh0 = wpool.tile([128, NCI, 512], bf, tag="wBh0",
                                      name="wkh0")
                    nc.scalar.dma_start(out=wkh0, in_=w_d["wk", 0][:, :, :])
                    nc.scalar.dma_start(out=m0_sb, in_=m0_d[:, :])
                    nc.gpsimd.dma_start(out=xkv, in_=xkvT_d[:, :, :])
                    wkh1 = wpool.tile([128, NCI, 512], bf, tag="wBh1",
                                      name="wkh1")
                    nc.gpsimd.dma_start(out=wkh1, in_=w_d["wk", 1][:, :, :])
                    wk = [[wkh0[:, ci, :], wkh1[:, ci, :]]
                          for ci in range(NCI)]
                    nc.gpsimd.dma_start(
                        out=bv_sb, in_=bv_d[0:1, :].broadcast_to((128, C)))

                    proj_qk(wq, xq, bq_sb, qT, psA, shpool,
                            filler=warm_filler)
                    # wv prefetch on sync ring (lands during Q/K compute)
                    wv = load_w("wv", nc.sync, "wV", "wv")
                    proj_qk(wk, xkv, bk_sb, kT, psA, shpool)

                # ---- phase 2+3 ----
                with tc.tile_pool(name="ypool", bufs=1) as ypool:
                  yT = ypool.tile([128, NCI, TP], bf, tag="yT")
                  with (
                    tc.tile_pool(name="ptp", bufs=2) as ptpool,
                    tc.tile_pool(name="qzp", bufs=1) as qzpool,
                    tc.tile_pool(name="psS", bufs=2, space="PSUM") as psS,
                    tc.tile_pool(name="stgp", bufs=2) as stgpool,
                    tc.tile_pool(name="rdp", bufs=1) as rdpool,
                    tc.tile_pool(name="ycp", bufs=2) as ycpool,
                  ):
                    # wp reuses wq's tags/buffers (dead after Q-proj)
                    wp = load_w("wp", nc.sync, "wA", "wp")
                    bp_sb = ypool.tile([128, C], f32, tag="bp")
                    nc.gpsimd.dma_start(
                        out=bp_sb, in_=bp_d[0:1, :].broadcast_to((128, C)))


                    qza = [qzpool.tile([128, TP], bf, tag=f"qza{i}",
                                       name=f"qza{i}")
                           for i in (0, 1)]
                    qzb = [qzpool.tile([128, TP], bf, tag=f"qzb{i}",
                                       name=f"qzb{i}")
                           for i in (0, 1)]
                    for i in (0, 1):
                        nc.vector.memset(qza[i][64:128, :], 0.0)
                        nc.vector.memset(qzb[i][0:64, :], 0.0)

                    def emit_qz(j):
                        nc.vector.tensor_copy(
                            qza[j % 2][0:64, :], qT[0:64, j, :])
                        nc.vector.tensor_copy(
                            qzb[j % 2][64:128, :], qT[64:128, j, :])

                    pts = {h: {} for h in range(H)}

                    def emit_st(h, nk):
                        j, par = h // 2, h % 2
                        qzt = qza[j % 2] if par == 0 else qzb[j % 2]
                        qlo, qhi, moff = R0SUB[nk]
                        ps = psS.tile([128, 1024], f32, tag="st",
                                      name=f"st{h}_{nk}")
                        nc.tensor.matmul(
                            ps[:, qlo:qhi],
                            kT[:, j, nk * 128:(nk + 1) * 128],
                            qzt[:, qlo:qhi], start=True, stop=True)
                        nc.tensor.matmul(
                            ps[:, 512:770],
                            kT[:, j, nk * 128:(nk + 1) * 128],
                            qzt[:, 512:770], start=True, stop=True)
                        pt = ptpool.tile([128, 772], bf, tag=f"pt{par}{nk}",
                                         name=f"pt{h}_{nk}")
                        nc.scalar.activation(
                            out=pt[:, qlo:770], in_=ps[:, qlo:770],
                            func=Exp, scale=0.125)
                        if moff is not None:
                            nc.gpsimd.tensor_mul(
                                pt[:, moff:moff + 128],
                                pt[:, moff:moff + 128], m0_sb)
                        pts[h][nk] = pt

                    def emit_st_tail(h):
                        j, par = h // 2, h % 2
                        qzt = qza[j % 2] if par == 0 else qzb[j % 2]
                        ps = psS.tile([128, 1024], f32, tag="st",
                                      name=f"st{h}_6")
                        nc.tensor.matmul(
                            ps[0:1, 0:258], kT[:, j, 768:769],
                            qzt[:, 512:770], start=True, stop=True)
                        pt6 = ptpool.tile([1, 772], bf, tag=f"pt{par}6",
                                          name=f"pt6_{h}")
                        nc.scalar.activation(
                            out=pt6[0:1, 513:770], in_=ps[0:1, 1:258],
                            func=Exp, scale=0.125)
                        nc.vector.tensor_copy(
                            pt6[0:1, 512:513], zbf[0:1, 0:1])
                        pts[h][6] = pt6

                    def emit_pv(h, nk, o):
                        vs = slice(h * (HD + 1), h * (HD + 1) + 128)
                        qlo, qhi, _ = R0SUB[nk]
                        nc.tensor.matmul(
                            o[:, qlo:qhi], vaug[:, nk, vs],
                            pts[h][nk][:, qlo:qhi],
                            start=(nk == 0), stop=False)
                        nc.tensor.matmul(
                            o[:, 512:770], vaug[:, nk, vs],
                            pts[h][nk][:, 512:770],
                            start=(nk == 0), stop=False)

                    def emit_pv_tail(h, o):
                        vs = slice(h * (HD + 1), h * (HD + 1) + 128)
                        nc.tensor.matmul(
                            o[:, 512:770], vaug[0:1, 6, vs],
                            pts[h][6][0:1, 512:770],
                            start=False, stop=True)

                    def emit_div_a(j, oa, ob):
                        # copy numerators+denominator rows out of PSUM in
                        # one wide DVE op per head — this alone releases
                        # psO for the next pair; everything downstream is
                        # off the PE's critical path.
                        yca = ycpool.tile([65, TP], bf, tag="yca",
                                          name=f"yca{j}")
                        ycb = ycpool.tile([65, TP], bf, tag="ycb",
                                          name=f"ycb{j}")
                        nc.vector.tensor_copy(yca[0:65, :], oa[0:65, 0:770])
                        nc.vector.tensor_copy(ycb[0:65, :], ob[0:65, 0:770])
                        return yca, ycb

                    def emit_div_b(j, yca, ycb):
                        # ones-row K=1 matmul broadcasts the raw denominator
                        # across 64 partitions; reciprocal on 64 DVE lanes
                        # straight out of PSUM; then the two division muls.
                        pba = psS.tile([128, 1024], f32, tag="st",
                                       name=f"pba{j}")
                        pbb = psS.tile([128, 1024], f32, tag="st",
                                       name=f"pbb{j}")
                        for (lo, hi) in (R0, R1):
                            nc.tensor.matmul(pba[0:64, lo:hi],
                                             ones128[64:65, 0:64],
                                             yca[64:65, lo:hi],
                                             start=True, stop=True)
                            nc.tensor.matmul(pbb[0:64, lo:hi],
                                             ones128[64:65, 0:64],
                                             ycb[64:65, lo:hi],
                                             start=True, stop=True)
                        ra = rdpool.tile([64, TP], f32, tag="ra",
                                         name=f"ra{j}")
                        rb = rdpool.tile([64, TP], f32, tag="rb",
                                         name=f"rb{j}")
                        nc.vector.reciprocal_approx_fast(
                            out=ra, in_=pba[0:64, 0:770])
                        nc.vector.reciprocal_approx_fast(
                            out=rb, in_=pbb[0:64, 0:770])
                        nc.vector.tensor_mul(
                            yT[0:64, j, :], yca[0:64, :], ra)
                        nc.vector.tensor_mul(
                            yT[64:128, j, :], ycb[0:64, :], rb)

                    emit_qz(0)
                    emit_qz(1)

                    # ---- V projection, interleaved with pair 0/1 S^T+exp
                    # (ACT gets a head start while the PE does V) ----
                    pre_units = []
                    for h in (0, 1, 2, 3):
                        for nk in range(6):
                            pre_units.append((h, nk))
                        pre_units.append((h, 6))

                    def emit_vgroup(g, wv):
                        pss = {}
                        for tt in g:
                            pss[tt] = psV.tile([128, 1024], f32, tag="psv",
                                               name=f"psv{tt}")
                        for ci in range(NCI):
                            for tt in g:
                                tsz = 128 if tt < 6 else 1
                                lhs = xkv[:, ci, tt * 128:tt * 128 + tsz]
                                for hf in (0, 1):
                                    nc.tensor.matmul(
                                        pss[tt][:tsz,
                                                hf * 512:hf * 512 + 512],
                                        lhs, wv[ci][hf],
                                        start=(ci == 0),
                                        stop=(ci == NCI - 1))
                        for tt in g:
                            tsz = 128 if tt < 6 else 1
                            va = vaug[:tsz, tt, 0:H * (HD + 1)].rearrange(
                                "p (h e) -> p h e", e=HD + 1)
                            nc.vector.tensor_add(
                                va[:, :, 0:HD],
                                pss[tt][:tsz, :].rearrange(
                                    "p (h d) -> p h d", h=H),
                                bv_sb[:tsz, :].rearrange(
                                    "p (h d) -> p h d", h=H))
                            nc.vector.tensor_copy(
                                va[:, :, HD:HD + 1],
                                ones16[:tsz, :].unsqueeze(2))
                            nc.vector.tensor_copy(
                                vaug[:tsz, tt, H * (HD + 1):VW],
                                zbf[:tsz, 0:VW - H * (HD + 1)])

                    with tc.tile_pool(name="psV", bufs=2,
                                      space="PSUM") as psV:
                        vgroups = [(0, 1), (2, 3), (4, 5), (6,)]
                        ui = 0
                        for gi, g in enumerate(vgroups):
                            emit_vgroup(g, wv)
                            n_units = 7 * (gi + 1)
                            while ui < min(n_units, len(pre_units)):
                                h, nk = pre_units[ui]
                                if nk == 6:
                                    emit_st_tail(h)
                                else:
                                    emit_st(h, nk)
                                ui += 1
                        while ui < len(pre_units):
                            h, nk = pre_units[ui]
                            if nk == 6:
                                emit_st_tail(h)
                            else:
                                emit_st(h, nk)
                            ui += 1

                    # ---- attention pairs ----
                    with tc.tile_pool(name="psO", bufs=1,
                                      space="PSUM") as psO:
                        pending = None  # previous pair's deferred division
                        for j in range(NCI):
                            a, bh = 2 * j, 2 * j + 1
                            oa = psO.tile([128, 1024], f32, tag="ova",
                                          name=f"ov{a}")
                            ob = psO.tile([128, 1024], f32, tag="ovb",
                                          name=f"ov{bh}")
                            for nk in range(6):
                                # S^T one nk ahead of PV so the exp+mask
                                # chain has a full nk of slack
                                if j >= 2:
                                    if nk < 5:
                                        emit_st(a, nk + 1)
                                        emit_st(bh, nk + 1)
                                    else:
                                        emit_st_tail(a)
                                        emit_st_tail(bh)
                                if nk == 2 and pending is not None:
                                    emit_div_b(*pending)
                                    pending = None
                                emit_pv(a, nk, oa)
                                emit_pv(bh, nk, ob)
                            # cross-pair lookahead: next pair's first S^T
                            # before this pair's tails, so ACT never idles
                            # across the pair boundary
                            if j + 1 >= 2 and j + 1 < NCI:
                                emit_st(a + 2, 0)
                                emit_st(bh + 2, 0)
                            emit_pv_tail(a, oa)
                            emit_pv_tail(bh, ob)
                            yca, ycb = emit_div_a(j, oa, ob)
                            pending = (j, yca, ycb)
                            # lookahead qz AFTER this pair's S^T emissions
                            # (correct WAR/RAW) and after the psO-releasing
                            # copies (DVE queue priority)
                            if j + 2 < NCI:
                                emit_qz(j + 2)
                        emit_div_b(*pending)

                  # ---- phase 3: output projection ----
                  with (
                    tc.tile_pool(name="psF", bufs=4, space="PSUM") as psF,
                    tc.tile_pool(name="opool", bufs=3) as opool,
                  ):
                    for g in (range(0, 4), range(4, 7)):
                        pss = {}
                        for tt in g:
                            pss[tt] = psF.tile([128, 1024], f32, tag="pso",
                                               name=f"pso{tt}")
                        for ci in range(NCI):
                            for tt in g:
                                tsz = 128 if tt < 6 else 1
                                lhs = yT[:, ci, tt * 128:tt * 128 + tsz]
                                for hf in (0, 1):
                                    nc.tensor.matmul(
                                        pss[tt][:tsz,
                                                hf * 512:hf * 512 + 512],
                                        lhs, wp[ci][hf],
                                        start=(ci == 0),
                                        stop=(ci == NCI - 1))
                        for tt in g:
                            tsz = 128 if tt < 6 else 1
                            ot = opool.tile([128, 1024], f32, tag="ot",
                                            name="ot")
                            nc.vector.tensor_add(
                                ot[:tsz, :], pss[tt][:tsz, :],
                                bp_sb[:tsz, :])
                            eng = nc.sync if tt % 2 == 0 else nc.scalar
                            eng.dma_start(
                                out=out_d[tt * 128:tt * 128 + tsz, :],
                                in_=ot[:tsz, :])

    nc.compile()
    return nc


def _host_prep(x_q, x_kv, rotary_pos_emb, Wq, bq, Wk, bk, Wv, bv, Wp, bp):
    import ml_dtypes
    f = np.float32
    bft = ml_dtypes.bfloat16
    x_q = np.asarray(x_q, f)
    x_kv = np.asarray(x_kv, f)
    freqs = np.asarray(rotary_pos_emb, f)

    # Even/odd pair-split permutation of the first 32 dims of each head, so
    # rotate_half becomes a 16-partition block swap on chip.
    perm = np.arange(C)
    for h in range(H):
        b0 = h * HD
        blk = np.empty(HD, np.int64)
        blk[0:16] = b0 + np.arange(0, 32, 2)
        blk[16:32] = b0 + np.arange(1, 32, 2)
        blk[32:64] = b0 + np.arange(32, 64)
        perm[b0:b0 + HD] = blk

    def wT(W, p=None):
        # -> two [128, NCI, 512] halves, contiguous per partition row
        W = np.asarray(W, f)
        if p is not None:
            W = W[p, :]
        Wt = W.T.reshape(NCI, 128, C).transpose(1, 0, 2)  # [128, NCI, C]
        return (np.ascontiguousarray(Wt[:, :, 0:512]).astype(bft),
                np.ascontiguousarray(Wt[:, :, 512:1024]).astype(bft))

    cosE = np.cos(freqs[:, 0::2]).T  # [16, T]
    cosO = np.cos(freqs[:, 1::2]).T
    sinE = -np.sin(freqs[:, 0::2]).T
    sinO = np.sin(freqs[:, 1::2]).T
    cosP = np.ones((128, TP), f)
    sinP = np.zeros((128, TP), f)
    for s in (0, 64):
        cosP[s:s + 16, :T] = cosE
        cosP[s + 16:s + 32, :T] = cosO
        sinP[s:s + 16, :T] = sinE
        sinP[s + 16:s + 32, :T] = sinO

    p_idx = np.arange(128)[:, None]
    f_idx = np.arange(128)[None, :]
    m0 = (p_idx < f_idx).astype(f)

    bqp = np.asarray(bq, f)[perm]
    bkp = np.asarray(bk, f)[perm]
    shared = {
        "bq2": np.ascontiguousarray(bqp.reshape(NCO, 128).T),
        "bk2": np.ascontiguousarray(bkp.reshape(NCO, 128).T),
        "bv1": np.asarray(bv, f).reshape(1, C).copy(),
        "bp1": np.asarray(bp, f).reshape(1, C).copy(),
        "cosP": np.ascontiguousarray(cosP).astype(bft),
        "sinP": np.ascontiguousarray(sinP).astype(bft),
        "m0": np.ascontiguousarray(m0).astype(bft),
    }
    for wn, W, p in (("wq", Wq, perm), ("wk", Wk, perm),
                     ("wv", Wv, None), ("wp", Wp, None)):
        h0, h1 = wT(W, p)
        shared[f"{wn}T0"] = h0
        shared[f"{wn}T1"] = h1

    def padT(xt):
        # [C, T] -> [128, NCI, TP] (partition-contiguous)
        out = np.zeros((C, TP), f)
        out[:, :T] = xt
        out = out.reshape(NCI, 128, TP).transpose(1, 0, 2)
        return np.ascontiguousarray(out).astype(bft)

    in_maps = []
    for b in range(B):
        m = dict(shared)
        m["xqT"] = padT(x_q[b].T)
        m["xkvT"] = padT(x_kv[b].T)
        in_maps.append(m)
    return in_maps


def kernel(x_q, x_kv, rotary_pos_emb, Wq, bq, Wk, bk, Wv, bv, Wp, bp):
    from concourse.bass_utils import run_bass_kernel_spmd

    if "nc" not in _CACHE:
        _CACHE["nc"] = _build_program()
    nc = _CACHE["nc"]

    in_maps = _host_prep(x_q, x_kv, rotary_pos_emb,
                         Wq, bq, Wk, bk, Wv, bv, Wp, bp)
    trace = os.environ.get("BTK_TRACE", "0") == "1"
    res = run_bass_kernel_spmd(
        nc, in_maps, core_ids=list(range(B)), trace=trace)
    _CACHE["last_result"] = res
    return np.stack([r["out"] for r in res.results], axis=0)
